# revision 33
# baseline (speedup 1.0000x reference)
"""Trainium2 Bass kernel for the GIN message-passing model (8 NeuronCores).

Sharding: graph partitioning.  Core c owns graphs [c*G/8, (c+1)*G/8) and the
contiguous node range of those graphs (batch is sorted), plus every edge whose
dst lands there (+ synthetic self-edges folding the GIN "+h" term into the
aggregation).  dst nodes get compact slot ranks.

Aggregation: edges are gathered with `dma_gather` (int16 indices, so the
source row space is split into 4 ranges => 4 passes).  Within a pass edges
are dst-sorted and packed into 128-position chunks aligned to 128-slot
"subbins"; a chunk's segment-sum is one matmul (gathered rows as stationary,
an on-chip-generated one-hot as moving operand) into the subbin's slice of a
512-slot "bin" PSUM bank.  Each (bin, pass) accumulates in PSUM, then one DVE
add folds it into the z accumulator in SBUF.  BatchNorm stats AllReduce; h1
is stored node-major (via PE transposes) and AllGathered for conv2's gather;
pooling is windowed one-hot matmuls; the MLP head runs feature-major.
"""

import sys

for _p in ("/opt/trn_rl_repo",):
    if _p not in sys.path:
        sys.path.insert(0, _p)

import numpy as np
from contextlib import ExitStack

import concourse.bass as bass
import concourse.bacc as bacc
import concourse.mybir as mybir
import concourse.tile as tile
from concourse.bass_utils import run_bass_kernel_spmd
from concourse.tile_rust import add_dep_helper

F32 = mybir.dt.float32
BF16 = mybir.dt.bfloat16
I32 = mybir.dt.int32
I16 = mybir.dt.int16
AF = mybir.ActivationFunctionType
ALU = mybir.AluOpType

BN_EPS = 1e-5
PADCOL = 200.0          # colidx value for pad positions (never matches 0..127)


class Cfg:
    def __init__(self, N=100000, E=500000, G=2048, D=128, OUT=64, FIN=2, W=8,
                 NR=4, NIMAX=8192, GW=32, GDT=BF16, DBG=99):
        self.N, self.E, self.G, self.D, self.OUT, self.FIN, self.W = N, E, G, D, OUT, FIN, W
        self.NR = NR        # source ranges (int16 index limit)
        self.NIMAX = NIMAX  # max positions per dma_gather
        self.GW = GW        # pooling window width (graphs)
        self.GDT = GDT      # gather dtype (bf16 or f32)
        self.DBG = DBG      # debug cut level (99 = full program)
        self.GPC = G // W   # graphs per core


DEFAULT_CFG = Cfg()


def _wrap_idx(lst):
    """dma_gather index layout: position j is read from row j%16, col j//16."""
    assert len(lst) % 16 == 0
    return np.tile(np.asarray(lst, np.int16).reshape(-1, 16).T, (8, 1))


# ---------------------------------------------------------------- host plan

def _plan(edge_index, batch, cfg):
    c = cfg
    batch = np.asarray(batch).astype(np.int64)
    ei = np.asarray(edge_index).astype(np.int64)
    owner = (batch // c.GPC).astype(np.int64)

    # self-edges appended
    src2 = np.concatenate([ei[0], np.arange(c.N, dtype=np.int64)])
    dst2 = np.concatenate([ei[1], np.arange(c.N, dtype=np.int64)])
    eowner = owner[dst2]

    # compact slot ranks per core
    n_real = np.zeros(c.W, np.int64)
    slot_of = np.full(c.N, -1, np.int64)
    node_lo = np.zeros(c.W + 1, np.int64)
    for ci in range(c.W):
        node_lo[ci] = np.searchsorted(batch, ci * c.GPC)
    node_lo[c.W] = c.N
    for ci in range(c.W):
        lo, hi = node_lo[ci], node_lo[ci + 1]
        n_real[ci] = hi - lo
        slot_of[lo:hi] = np.arange(hi - lo)
    S = int(((n_real.max() + 511) // 512) * 512)
    assert 2 * S <= 32767, f"S={S} too large for int16 conv2 ranges"
    nbin = S // 512
    nsub = S // 128
    nSC = S // 128
    gslot = owner * S + slot_of

    def build_conv(src_row, R):
        """src_row: per-edge source row id in the gather table (size R).
        Ranges are interleaved (pass = row % NR) so per-core locality in the
        source space cannot overload one pass.  The device view is
        table.rearrange("(q four) f -> four q f")[r] with elem_step.
        Returns common chunk structure + per-core idx/colidx arrays."""
        RSZ = -(-max(R, 1) // c.NR)
        RSZ = ((RSZ + 127) // 128) * 128
        assert RSZ <= 32767
        epass = src_row % c.NR
        # per (core, pass, subbin) edge lists
        counts = np.zeros((c.W, c.NR, nsub), np.int64)
        percore_edges = []
        for ci in range(c.W):
            m = eowner == ci
            sl = slot_of[dst2[m]]
            pr = epass[m]
            rows = src_row[m]
            sub = sl // 128
            order = np.lexsort((sl, sub, pr))
            sl, pr, rows, sub = sl[order], pr[order], rows[order], sub[order]
            np.add.at(counts[ci], (pr, sub), 1)
            percore_edges.append((sl, pr, rows, sub))
        # common chunk structure
        nch = np.maximum(1, -(-counts.max(axis=0) // 128))   # [NR, nsub]
        chunks = []     # (pass, subbin)
        seg_of = {}
        for r in range(c.NR):
            for sb in range(nsub):
                seg_of[(r, sb)] = (len(chunks), int(nch[r, sb]))
                for k in range(int(nch[r, sb])):
                    chunks.append((r, sb))
        C = len(chunks)
        POS = C * 128
        pass_pos_lo = np.zeros(c.NR + 1, np.int64)
        for r in range(c.NR):
            pass_pos_lo[r + 1] = pass_pos_lo[r] + 128 * int(nch[r].sum())
        # per-core arrays
        cores = []
        for ci in range(c.W):
            sl, pr, rows, sub = percore_edges[ci]
            idx_local = np.zeros(POS, np.int64)          # pad -> row 0 of range
            colv = np.full((128, C), PADCOL, np.float64)
            # compute position of each edge: within its (pass, subbin) segment
            seg_base = {}
            cursor = {}
            pos = 0
            for r in range(c.NR):
                for sb in range(nsub):
                    seg_base[(r, sb)] = pos
                    cursor[(r, sb)] = 0
                    pos += 128 * int(nch[r, sb])
            # vectorized-ish placement
            key = pr * nsub + sub
            # edges are sorted by (pr, sub, sl); within segment consecutive
            uniq, start_idx = np.unique(key, return_index=True)
            end_idx = np.append(start_idx[1:], len(key))
            for u, s0, s1 in zip(uniq, start_idx, end_idx):
                r, sb = int(u) // nsub, int(u) % nsub
                base = seg_base[(r, sb)]
                n = s1 - s0
                p = base + np.arange(n)
                idx_local[p] = rows[s0:s1] // c.NR
                colv[p % 128, p // 128] = sl[s0:s1] - sb * 128
            # per-pass wrapped idx arrays, concatenated into [128, POS/16]
            wrapped = [
                _wrap_idx(idx_local[pass_pos_lo[r]:pass_pos_lo[r + 1]])
                for r in range(c.NR) if pass_pos_lo[r + 1] > pass_pos_lo[r]
            ]
            idx16 = np.concatenate(wrapped, axis=1) if wrapped else np.zeros((128, 0), np.int16)
            cores.append(dict(idx16=idx16, colidx=colv))
        # gather op list: per pass, ops of <= NIMAX positions
        ops = []        # (pass, pos_lo, ni)
        for r in range(c.NR):
            p0, p1 = int(pass_pos_lo[r]), int(pass_pos_lo[r + 1])
            while p0 < p1:
                ni = min(c.NIMAX, p1 - p0)
                ops.append((r, p0, ni))
                p0 += ni
        return dict(R=R, RSZ=RSZ, C=C, POS=POS, chunks=chunks, ops=ops,
                    cores=cores, seg_of=seg_of)

    conv1 = build_conv(src2, c.N)                # gather from x rows
    conv2 = build_conv(gslot[src2], c.W * S)     # gather from h1all rows
    assert conv2["R"] <= c.W * S

    # pooling plan
    gos_all = []
    for ci in range(c.W):
        gos = np.full(S, -1, np.int64)
        lo, hi = node_lo[ci], node_lo[ci + 1]
        gos[:hi - lo] = batch[lo:hi] - ci * c.GPC
        gos_all.append(gos)
    win_lo = np.zeros(nSC, np.int64)
    prev = 0
    for k in range(nSC):
        lo_k, hi_k = c.GPC, -1
        for gos in gos_all:
            seg = gos[k * 128:(k + 1) * 128]
            v = seg[seg >= 0]
            if len(v):
                lo_k = min(lo_k, int(v.min()))
                hi_k = max(hi_k, int(v.max()))
        if hi_k < 0:
            lo_k = hi_k = min(prev, c.GPC - 1)
        assert hi_k - lo_k + 1 <= c.GW, f"pool window too wide: {lo_k}..{hi_k}"
        lo_k = max(0, min(lo_k, c.GPC - c.GW))
        assert lo_k <= prev + c.GW, "pool window coverage gap"
        win_lo[k] = lo_k
        prev = max(prev, lo_k + c.GW - 1)
    covered = np.zeros(c.GPC, bool)
    for k in range(nSC):
        covered[win_lo[k]:win_lo[k] + c.GW] = True
    assert covered.all()

    pmats = []
    for ci in range(c.W):
        pmat = np.zeros((128, nSC * c.GW), np.float32)
        gos = gos_all[ci]
        for k in range(nSC):
            seg = gos[k * 128:(k + 1) * 128]
            for p in range(128):
                if seg[p] >= 0:
                    w = int(seg[p] - win_lo[k])
                    pmat[p, k * c.GW + w] = 1.0
        pmats.append(pmat)

    return dict(S=S, nbin=nbin, nSC=nSC, win_lo=win_lo, conv=[conv1, conv2],
                n_real=n_real, pmats=pmats)


# ---------------------------------------------------------------- program

def _build(plan, cfg):
    c = cfg
    S, nbin, nSC = plan["S"], plan["nbin"], plan["nSC"]
    win_lo = plan["win_lo"]
    D, OUT, FIN, GPC = c.D, c.OUT, c.FIN, c.GPC
    rg = [list(range(c.W))]
    nG = S // 512
    GDT = c.GDT

    nc = bacc.Bacc(num_devices=c.W)

    # ---- external inputs
    xg_d = nc.dram_tensor("xg", [plan["conv"][0]["RSZ"] * c.NR, D], GDT,
                          kind="ExternalInput")
    pmat_d = nc.dram_tensor("pmat", [128, nSC * c.GW], F32, kind="ExternalInput")
    idx_d, col_d = [], []
    for li in (0, 1):
        cv = plan["conv"][li]
        idx_d.append(nc.dram_tensor(f"idx{li}", [128, cv["POS"] // 16], I16,
                                    kind="ExternalInput"))
        col_d.append(nc.dram_tensor(f"col{li}", [128, cv["C"]], GDT,
                                    kind="ExternalInput"))
    code_d = nc.dram_tensor("code", [GPC, D], F32, kind="ExternalInput")
    ident_d = nc.dram_tensor("ident", [128, 128], F32, kind="ExternalInput")
    nh_d = nc.dram_tensor("nh", [128, 1], F32, kind="ExternalInput")

    wspec = {
        "c1_w1": [D, D], "c1_b1": [D], "c1_gamma": [D], "c1_beta": [D],
        "c1_w2": [D, D], "c1_b2": [D],
        "c2_w1": [D, D], "c2_b1": [D], "c2_gamma": [D], "c2_beta": [D],
        "c2_w2": [D, D], "c2_b2": [D],
        "g_l1_w": [D, D], "g_l1_b": [D], "g_l2_w": [D, OUT], "g_l2_b": [OUT],
        "fc1_w": [D, D], "fc1_b": [D], "fc2_w": [D, D], "fc2_b": [D],
        "fc3_w": [D, OUT], "fc3_b": [OUT],
        "fin_w": [2 * OUT, FIN], "fin_b": [FIN],
    }
    wd = {k: nc.dram_tensor(k, v, F32, kind="ExternalInput") for k, v in wspec.items()}

    out_d = nc.dram_tensor("out", [FIN, GPC], F32, kind="ExternalOutput")

    # ---- internal DRAM
    h1loc_d = nc.dram_tensor("h1loc", [S, D], GDT)
    RSZ2 = plan["conv"][1]["RSZ"]
    h1all_d = nc.dram_tensor("h1all", [RSZ2 * c.NR, D], GDT, addr_space="Shared")
    ar_in = [nc.dram_tensor(f"ar{i}i", [128, 2], F32) for i in (1, 2)]
    ar_out = [nc.dram_tensor(f"ar{i}o", [128, 2], F32, addr_space="Shared")
              for i in (1, 2)]

    with tile.TileContext(nc) as tc, ExitStack() as ctx:
        const = ctx.enter_context(tc.tile_pool(name="const", bufs=1))
        work = ctx.enter_context(tc.tile_pool(name="work", bufs=3))
        wide = ctx.enter_context(tc.tile_pool(name="wide", bufs=1))
        pp = ctx.enter_context(tc.tile_pool(name="pp", bufs=2, space="PSUM"))
        pp3 = ctx.enter_context(tc.tile_pool(name="pp3", bufs=3, space="PSUM"))

        def cload(dram_ap, shape, dtype, tag):
            t = const.tile(shape, dtype, tag=tag)
            nc.sync.dma_start(out=t[:], in_=dram_ap)
            return t

        ident_s = cload(ident_d[:], [128, 128], F32, "ident")
        nh_s = cload(nh_d[:], [128, 1], F32, "nh")
        pmat_s = cload(pmat_d[:], [128, nSC * c.GW], F32, "pmat")

        ws = {}
        for k, shp in wspec.items():
            if len(shp) == 2:
                ws[k] = cload(wd[k][:], shp, F32, k)
            else:
                ws[k] = cload(wd[k][:, None], [shp[0], 1], F32, k)
        finw_hi = const.tile([OUT, FIN], F32, tag="finw_hi")
        nc.sync.dma_start(out=finw_hi[:], in_=wd["fin_w"][OUT:2 * OUT, :])

        # iota row pattern repeated (for one-hot gen), in gather dtype
        IOB = 8  # chunks per one-hot op
        iota_i = const.tile([128, IOB * 128], I32, tag="iota_i")
        nc.gpsimd.iota(iota_i[:], pattern=[[0, IOB], [1, 128]], base=0,
                       channel_multiplier=0)
        iota_s = const.tile([128, IOB * 128], GDT, tag="iota_s")
        nc.vector.tensor_copy(out=iota_s[:], in_=iota_i[:])

        ones_d1 = const.tile([OUT, 1], F32, tag="ones_d1")
        nc.vector.memset(ones_d1[:], 1.0)
        ones_1d = const.tile([1, OUT], F32, tag="ones_1d")
        nc.vector.memset(ones_1d[:], 1.0)
        ones_f1 = const.tile([FIN, 1], F32, tag="ones_f1")
        nc.vector.memset(ones_f1[:], 1.0)
        ones_1f = const.tile([1, FIN], F32, tag="ones_1f")
        nc.vector.memset(ones_1f[:], 1.0)

        # =========================== code MLP branch (fills bubbles)
        nbl = (GPC + 127) // 128
        code_nm = const.tile([128, nbl * D], F32, tag="code_nm")
        nc.sync.dma_start(
            out=code_nm[:].rearrange("p (b f) -> p b f", b=nbl),
            in_=code_d[:].rearrange("(b p) f -> p b f", p=128))
        codeT = const.tile([128, GPC], F32, tag="codeT")
        for b in range(nbl):
            tp = pp.tile([128, 128], F32, tag="tp")
            nc.tensor.transpose(out=tp[:], in_=code_nm[:, b * D:(b + 1) * D],
                                identity=ident_s[:])
            nc.vector.tensor_copy(out=codeT[:, b * 128:(b + 1) * 128], in_=tp[:])
        cps = pp3.tile([128, GPC], F32, tag="zp")
        nc.tensor.matmul(out=cps[:], lhsT=ws["fc1_w"][:], rhs=codeT[:],
                         start=True, stop=True)
        c1_s = const.tile([128, GPC], F32, tag="c1_s")
        nc.scalar.activation(out=c1_s[:], in_=cps[:], func=AF.Relu,
                             bias=ws["fc1_b"][:, :1])
        cps2 = pp3.tile([128, GPC], F32, tag="zp")
        nc.tensor.matmul(out=cps2[:], lhsT=ws["fc2_w"][:], rhs=c1_s[:],
                         start=True, stop=True)
        c2_s = const.tile([128, GPC], F32, tag="c2_s")
        nc.scalar.activation(out=c2_s[:], in_=cps2[:], func=AF.Relu,
                             bias=ws["fc2_b"][:, :1])
        cps3 = pp.tile([OUT, GPC], F32, tag="up")
        nc.tensor.matmul(out=cps3[:], lhsT=ws["fc3_w"][:], rhs=c2_s[:],
                         start=True, stop=True)
        c3_s = const.tile([OUT, GPC], F32, tag="c3_s")
        nc.scalar.activation(out=c3_s[:], in_=cps3[:], func=AF.Identity,
                             bias=ws["fc3_b"][:, :1])
        e64 = const.tile([OUT, GPC], F32, tag="e64")
        nc.scalar.activation(out=e64[:], in_=c3_s[:], func=AF.Exp)
        lsp = pp.tile([1, GPC], F32, tag="tp")
        nc.tensor.matmul(out=lsp[:], lhsT=ones_d1[:], rhs=e64[:],
                         start=True, stop=True)
        lse_s = const.tile([1, GPC], F32, tag="lse_s")
        nc.scalar.activation(out=lse_s[:], in_=lsp[:], func=AF.Ln)
        bcp = pp.tile([OUT, GPC], F32, tag="up")
        nc.tensor.matmul(out=bcp[:], lhsT=ones_1d[:], rhs=lse_s[:],
                         start=True, stop=True)
        code_embT = const.tile([OUT, GPC], F32, tag="code_embT")
        nc.vector.tensor_tensor(out=code_embT[:], in0=c3_s[:], in1=bcp[:],
                                op=ALU.subtract)

        # =========================== GIN convs
        zu_t = wide.tile([128, S], F32, tag="zu")     # z, then u, then zb (in place)
        pooled_acc = const.tile([128, GPC], F32, tag="pooled_acc")
        nc.vector.memset(pooled_acc[:], 0.0)
        ag_inst = None

        idxcol = {}
        for li, cv_ in enumerate(plan["conv"]):
            i_s = const.tile([128, cv_["POS"] // 16], I16, tag=f"idx{li+1}")
            nc.sync.dma_start(out=i_s[:], in_=idx_d[li][:])
            c_s = const.tile([128, cv_["C"]], GDT, tag=f"col{li+1}")
            nc.sync.dma_start(out=c_s[:], in_=col_d[li][:])
            idxcol[li + 1] = (i_s, c_s)

        def conv(idx, cv, src_dram, idx_dram, col_dram,
                 w1_s, b1_s, gam_s, bet_s, w2_s, b2_s, ari, aro, dep=None,
                 upto="full"):
            C, POS = cv["C"], cv["POS"]
            chunks, ops = cv["chunks"], cv["ops"]
            idx_s, col_s = idxcol[idx]
            ssum = const.tile([128, nG], F32, tag=f"ssum{idx}")
            ssq = const.tile([128, nG], F32, tag=f"ssq{idx}")

            # map chunk -> (op index, block within op)
            chunk_op = []
            for oi, (r, plo, ni) in enumerate(ops):
                for b in range(ni // 128):
                    chunk_op.append((oi, b))
            assert len(chunk_op) == C

            gtiles = {}
            stiles = {}
            cur_group = None       # (bin, pass)
            zp = None
            group_started = set()  # bins with first (copy) group done

            def close_group():
                nonlocal cur_group, zp
                if cur_group is None:
                    return
                bn = cur_group[0]
                cols = slice(bn * 512, (bn + 1) * 512)
                if bn in group_started:
                    nc.vector.tensor_tensor(out=zu_t[:, cols], in0=zu_t[:, cols],
                                            in1=zp[:], op=ALU.add)
                else:
                    nc.vector.tensor_copy(out=zu_t[:, cols], in_=zp[:])
                    group_started.add(bn)
                cur_group, zp = None, None

            for ci in range(C):
                r, sb = chunks[ci]
                bn, sl4 = sb // 4, sb % 4
                oi, blk = chunk_op[ci]
                if oi not in gtiles:
                    opr, plo, ni = ops[oi]
                    gt = work.tile([128, c.NIMAX], GDT, tag="gt")
                    src_view = src_dram[:].rearrange(
                        "(q four) f -> four q f", four=c.NR)[opr]
                    g_ins = nc.gpsimd.dma_gather(
                        gt[:, :ni].rearrange("p (k f) -> p k f", k=ni // 128),
                        src_view,
                        idx_s[:, plo // 16:(plo + ni) // 16],
                        ni, ni, 128, elem_step=c.NR * D,
                        single_packet=False)
                    if dep is not None:
                        add_dep_helper(g_ins.ins, dep.ins, True, "gather after AG")
                    gtiles = {oi: gt}
                if ci % IOB == 0:
                    nob = min(IOB, C - ci)
                    st = work.tile([128, IOB * 128], GDT, tag="st")
                    nc.vector.tensor_tensor(
                        out=st[:, :nob * 128].rearrange("p (c f) -> p c f", c=nob),
                        in0=col_s[:, ci:ci + nob].to_broadcast([128, nob, 128]),
                        in1=iota_s[:, :nob * 128].rearrange("p (c f) -> p c f", c=nob),
                        op=ALU.is_equal)
                    stiles = {ci // IOB: st}
                if cur_group != (bn, r):
                    close_group()
                    cur_group = (bn, r)
                    zp = pp3.tile([128, 512], F32, tag="zp")
                # start flag: first chunk of this (bin, pass) group
                is_first = (ci == 0 or chunks[ci - 1][0] != r
                            or chunks[ci - 1][1] // 4 != bn)
                is_last = (ci == C - 1 or chunks[ci + 1][0] != chunks[ci][0]
                           or chunks[ci + 1][1] // 4 != bn)
                nc.tensor.matmul(
                    out=zp[:, sl4 * 128:(sl4 + 1) * 128],
                    lhsT=gtiles[oi][:, blk * 128:(blk + 1) * 128],
                    rhs=stiles[ci // IOB][:, (ci % IOB) * 128:(ci % IOB + 1) * 128],
                    start=is_first, stop=is_last,
                    skip_group_check=True)
            close_group()
            if upto == "agg":
                return

            # ---- layer 1 + stats
            for g in range(nG):
                cols = slice(g * 512, (g + 1) * 512)
                up = pp.tile([128, 512], F32, tag="up")
                nc.tensor.matmul(out=up[:], lhsT=w1_s[:], rhs=zu_t[:, cols],
                                 start=True, stop=True)
                nc.scalar.activation(out=zu_t[:, cols], in_=up[:],
                                     func=AF.Identity, bias=b1_s[:, :1],
                                     accum_out=ssum[:, g:g + 1])
                sq = work.tile([128, 512], F32, tag="sq")
                nc.scalar.activation(out=sq[:], in_=zu_t[:, cols],
                                     func=AF.Square,
                                     accum_out=ssq[:, g:g + 1])

            # ---- BN stats + AllReduce
            sum_r = const.tile([128, 1], F32, tag=f"sum_r{idx}")
            ssq_r = const.tile([128, 1], F32, tag=f"ssq_r{idx}")
            nc.vector.tensor_reduce(out=sum_r[:], in_=ssum[:],
                                    axis=mybir.AxisListType.X, op=ALU.add)
            nc.vector.tensor_reduce(out=ssq_r[:], in_=ssq[:],
                                    axis=mybir.AxisListType.X, op=ALU.add)
            b1sq = const.tile([128, 1], F32, tag=f"b1sq{idx}")
            nc.scalar.activation(out=b1sq[:], in_=b1_s[:], func=AF.Square)
            tmp1 = const.tile([128, 1], F32, tag=f"tmp1_{idx}")
            nc.vector.tensor_tensor(out=tmp1[:], in0=b1_s[:], in1=nh_s[:],
                                    op=ALU.mult)
            nc.vector.tensor_tensor(out=sum_r[:], in0=sum_r[:], in1=tmp1[:],
                                    op=ALU.subtract)
            nc.vector.tensor_tensor(out=tmp1[:], in0=b1sq[:], in1=nh_s[:],
                                    op=ALU.mult)
            nc.vector.tensor_tensor(out=ssq_r[:], in0=ssq_r[:], in1=tmp1[:],
                                    op=ALU.subtract)
            if upto == "stats":
                return
            pack = const.tile([128, 2], F32, tag=f"pack{idx}")
            nc.vector.tensor_copy(out=pack[:, 0:1], in_=sum_r[:])
            nc.vector.tensor_copy(out=pack[:, 1:2], in_=ssq_r[:])
            nc.sync.dma_start(out=ari[:], in_=pack[:])
            ar = nc.gpsimd.collective_compute(
                "AllReduce", ALU.add, replica_groups=rg,
                ins=[ari[:]], outs=[aro[:]])
            rb = const.tile([128, 2], F32, tag=f"rb{idx}")
            d = nc.sync.dma_start(out=rb[:], in_=aro[:])
            add_dep_helper(d.ins, ar.ins, True, "read after AR")
            mean = const.tile([128, 1], F32, tag=f"mean{idx}")
            m2 = const.tile([128, 1], F32, tag=f"m2{idx}")
            nc.scalar.activation(out=mean[:], in_=rb[:, 0:1], func=AF.Copy,
                                 scale=1.0 / c.N)
            nc.scalar.activation(out=m2[:], in_=rb[:, 1:2], func=AF.Copy,
                                 scale=1.0 / c.N)
            msq = const.tile([128, 1], F32, tag=f"msq{idx}")
            nc.scalar.activation(out=msq[:], in_=mean[:], func=AF.Square)
            var = const.tile([128, 1], F32, tag=f"var{idx}")
            nc.vector.tensor_tensor(out=var[:], in0=m2[:], in1=msq[:],
                                    op=ALU.subtract)
            nc.vector.tensor_scalar_add(out=var[:], in0=var[:], scalar1=BN_EPS)
            std = const.tile([128, 1], F32, tag=f"std{idx}")
            nc.scalar.activation(out=std[:], in_=var[:], func=AF.Sqrt)
            inv = const.tile([128, 1], F32, tag=f"inv{idx}")
            nc.vector.reciprocal(out=inv[:], in_=std[:])
            sc = const.tile([128, 1], F32, tag=f"sc{idx}")
            nc.vector.tensor_tensor(out=sc[:], in0=gam_s[:], in1=inv[:],
                                    op=ALU.mult)
            sh = const.tile([128, 1], F32, tag=f"sh{idx}")
            nc.vector.tensor_tensor(out=sh[:], in0=mean[:], in1=sc[:],
                                    op=ALU.mult)
            nc.vector.tensor_tensor(out=sh[:], in0=bet_s[:], in1=sh[:],
                                    op=ALU.subtract)
            if upto == "bn":
                return

            # ---- BN apply + relu (in place), layer 2, transposes
            for g in range(nG):
                cols = slice(g * 512, (g + 1) * 512)
                nc.scalar.activation(out=zu_t[:, cols], in_=zu_t[:, cols],
                                     func=AF.Relu, bias=sh[:, :1],
                                     scale=sc[:, :1])
                hp = pp.tile([128, 512], F32, tag="up")
                nc.tensor.matmul(out=hp[:], lhsT=w2_s[:], rhs=zu_t[:, cols],
                                 start=True, stop=True)
                hb = work.tile([128, 512], F32, tag="hb")
                nc.scalar.activation(out=hb[:], in_=hp[:], func=AF.Relu,
                                     bias=b2_s[:, :1])
                hnm = work.tile([128, 4 * D], GDT if idx == 1 else F32, tag="hnm")
                for t in range(4):
                    tp = pp.tile([128, 128], F32, tag="tp")
                    nc.tensor.transpose(out=tp[:], in_=hb[:, t * 128:(t + 1) * 128],
                                        identity=ident_s[:])
                    nc.vector.tensor_copy(out=hnm[:, t * D:(t + 1) * D], in_=tp[:])
                    if idx == 2:
                        k = g * 4 + t
                        lo = int(win_lo[k])
                        poolw = pp.tile([128, c.GW], F32, tag="tp")
                        nc.tensor.matmul(
                            out=poolw[:],
                            lhsT=hnm[:, t * D:(t + 1) * D],
                            rhs=pmat_s[:, k * c.GW:(k + 1) * c.GW],
                            start=True, stop=True)
                        nc.vector.tensor_tensor(
                            out=pooled_acc[:, lo:lo + c.GW],
                            in0=pooled_acc[:, lo:lo + c.GW],
                            in1=poolw[:], op=ALU.add)
                if idx == 1:
                    nc.sync.dma_start(
                        out=h1loc_d[g * 512:(g + 1) * 512, :].rearrange(
                            "(b p) f -> p b f", p=128),
                        in_=hnm[:].rearrange("p (b f) -> p b f", b=4))

        cvs = plan["conv"]
        dbg = c.DBG
        upto1 = {1: "agg", 2: "stats", 3: "bn"}.get(dbg, "full")
        conv(1, cvs[0], xg_d, idx_d[0], col_d[0],
             ws["c1_w1"], ws["c1_b1"], ws["c1_gamma"], ws["c1_beta"],
             ws["c1_w2"], ws["c1_b2"], ar_in[0], ar_out[0], upto=upto1)
        if dbg >= 5:
            ag_inst = nc.gpsimd.collective_compute(
                "AllGather", ALU.bypass, replica_groups=rg,
                ins=[h1loc_d[:]], outs=[h1all_d[:]])
        if dbg >= 6:
            # conv2 gathers must run after the AllGather lands
            conv(2, cvs[1], h1all_d, idx_d[1], col_d[1],
                 ws["c2_w1"], ws["c2_b1"], ws["c2_gamma"], ws["c2_beta"],
                 ws["c2_w2"], ws["c2_b2"], ar_in[1], ar_out[1], dep=ag_inst)
        if dbg < 99:
            pout = const.tile([FIN, GPC], F32, tag="outT")
            nc.vector.tensor_copy(out=pout[:], in_=zu_t[0:FIN, 0:GPC])
            nc.sync.dma_start(out=out_d[:], in_=pout[:])
        else:
            # =========================== head
            hd1 = pp3.tile([128, GPC], F32, tag="zp")
            nc.tensor.matmul(out=hd1[:], lhsT=ws["g_l1_w"][:], rhs=pooled_acc[:],
                             start=True, stop=True)
            t_s = const.tile([128, GPC], F32, tag="t_s")
            nc.scalar.activation(out=t_s[:], in_=hd1[:], func=AF.Relu,
                                 bias=ws["g_l1_b"][:, :1])
            hd2 = pp.tile([OUT, GPC], F32, tag="up")
            nc.tensor.matmul(out=hd2[:], lhsT=ws["g_l2_w"][:], rhs=t_s[:],
                             start=True, stop=True)
            trans_embT = const.tile([OUT, GPC], F32, tag="trans_embT")
            nc.scalar.activation(out=trans_embT[:], in_=hd2[:], func=AF.Identity,
                                 bias=ws["g_l2_b"][:, :1])
            fp = pp.tile([FIN, GPC], F32, tag="tp")
            nc.tensor.matmul(out=fp[:], lhsT=ws["fin_w"][0:OUT, :],
                             rhs=code_embT[:], start=True, stop=False,
                             skip_group_check=True)
            nc.tensor.matmul(out=fp[:], lhsT=finw_hi[:],
                             rhs=trans_embT[:], start=False, stop=True,
                             skip_group_check=True)
            f_s = const.tile([FIN, GPC], F32, tag="f_s")
            nc.scalar.activation(out=f_s[:], in_=fp[:], func=AF.Identity,
                                 bias=ws["fin_b"][:, :1])
            ef = const.tile([FIN, GPC], F32, tag="ef")
            nc.scalar.activation(out=ef[:], in_=f_s[:], func=AF.Exp)
            lfp = pp.tile([1, GPC], F32, tag="up")
            nc.tensor.matmul(out=lfp[:], lhsT=ones_f1[:], rhs=ef[:],
                             start=True, stop=True)
            lf_s = const.tile([1, GPC], F32, tag="lf_s")
            nc.scalar.activation(out=lf_s[:], in_=lfp[:], func=AF.Ln)
            bfp = pp3.tile([FIN, GPC], F32, tag="zp")
            nc.tensor.matmul(out=bfp[:], lhsT=ones_1f[:], rhs=lf_s[:],
                             start=True, stop=True)
            outT = const.tile([FIN, GPC], F32, tag="outT")
            nc.vector.tensor_tensor(out=outT[:], in0=f_s[:], in1=bfp[:],
                                    op=ALU.subtract)
            nc.sync.dma_start(out=out_d[:], in_=outT[:])

    # order conv2 gathers after the AllGather
    if not nc.is_finalized():
        nc.finalize()
    return nc


# ---------------------------------------------------------------- runner

def make_in_maps(inputs, plan, cfg):
    c = cfg
    wnames = ["c1_w1", "c1_b1", "c1_gamma", "c1_beta", "c1_w2", "c1_b2",
              "c2_w1", "c2_b1", "c2_gamma", "c2_beta", "c2_w2", "c2_b2",
              "g_l1_w", "g_l1_b", "g_l2_w", "g_l2_b",
              "fc1_w", "fc1_b", "fc2_w", "fc2_b", "fc3_w", "fc3_b",
              "fin_w", "fin_b"]
    np_gdt = np.float32 if c.GDT == F32 else __import__("ml_dtypes").bfloat16
    x = np.asarray(inputs["x"], np.float32)
    R1, RSZ1 = plan["conv"][0]["R"], plan["conv"][0]["RSZ"]
    xg = np.zeros((RSZ1 * c.NR, c.D), np_gdt)
    xg[:x.shape[0]] = x.astype(np_gdt)
    code = np.ascontiguousarray(np.asarray(inputs["code_x"], np.float32))
    ident = np.eye(128, dtype=np.float32)
    in_maps = []
    for ci in range(c.W):
        m = {
            "xg": xg,
            "pmat": plan["pmats"][ci],
            "code": code[ci * c.GPC:(ci + 1) * c.GPC],
            "ident": ident,
            "nh": np.full((128, 1), float(plan["S"] - plan["n_real"][ci]),
                          np.float32),
        }
        for li in (0, 1):
            cv = plan["conv"][li]
            m[f"idx{li}"] = cv["cores"][ci]["idx16"]
            m[f"col{li}"] = cv["cores"][ci]["colidx"].astype(np_gdt)
        for k in wnames:
            m[k] = np.ascontiguousarray(np.asarray(inputs[k], np.float32))
        in_maps.append(m)
    return in_maps


_CACHE = {}


def _get_compiled(inputs, cfg):
    if "prog" not in _CACHE:
        plan = _plan(inputs["edge_index"], inputs["batch"], cfg)
        nc = _build(plan, cfg)
        _CACHE["prog"] = (plan, nc)
    return _CACHE["prog"]


def kernel(**inputs) -> np.ndarray:
    cfg = DEFAULT_CFG
    plan, nc = _get_compiled(inputs, cfg)
    in_maps = make_in_maps(inputs, plan, cfg)
    res = run_bass_kernel_spmd(nc, in_maps, core_ids=list(range(cfg.W)))
    outs = [res.results[ci]["out"].T for ci in range(cfg.W)]
    return np.ascontiguousarray(np.concatenate(outs, axis=0).astype(np.float32))


# revision 34
# speedup vs baseline: 1.5064x; 1.5064x over previous
"""Trainium2 Bass kernel for the GIN message-passing model (8 NeuronCores).

Sharding: graph partitioning.  Core c owns graphs [c*G/8, (c+1)*G/8) and the
contiguous node range of those graphs (batch is sorted), plus every edge whose
dst lands there (+ synthetic self-edges folding the GIN "+h" term into the
aggregation).  dst nodes get compact slot ranks.

Aggregation: edges are gathered with `dma_gather` (int16 indices, so the
source row space is split into 4 ranges => 4 passes).  Within a pass edges
are dst-sorted and packed into 128-position chunks aligned to 128-slot
"subbins"; a chunk's segment-sum is one matmul (gathered rows as stationary,
an on-chip-generated one-hot as moving operand) into the subbin's slice of a
512-slot "bin" PSUM bank.  Each (bin, pass) accumulates in PSUM, then one DVE
add folds it into the z accumulator in SBUF.  BatchNorm stats AllReduce; h1
is stored node-major (via PE transposes) and AllGathered for conv2's gather;
pooling is windowed one-hot matmuls; the MLP head runs feature-major.
"""

import sys

for _p in ("/opt/trn_rl_repo",):
    if _p not in sys.path:
        sys.path.insert(0, _p)

import numpy as np
from contextlib import ExitStack

import concourse.bass as bass
import concourse.bacc as bacc
import concourse.mybir as mybir
import concourse.tile as tile
from concourse.bass_utils import run_bass_kernel_spmd
from concourse.tile_rust import add_dep_helper

F32 = mybir.dt.float32
BF16 = mybir.dt.bfloat16
I32 = mybir.dt.int32
I16 = mybir.dt.int16
AF = mybir.ActivationFunctionType
ALU = mybir.AluOpType

BN_EPS = 1e-5
PADCOL = 200.0          # colidx value for pad positions (never matches 0..127)


class Cfg:
    def __init__(self, N=100000, E=500000, G=2048, D=128, OUT=64, FIN=2, W=8,
                 NR=4, NIMAX=4096, GW=32, GDT=BF16, DBG=99):
        self.N, self.E, self.G, self.D, self.OUT, self.FIN, self.W = N, E, G, D, OUT, FIN, W
        self.NR = NR        # source ranges (int16 index limit)
        self.NIMAX = NIMAX  # max positions per dma_gather
        self.GW = GW        # pooling window width (graphs)
        self.GDT = GDT      # gather dtype (bf16 or f32)
        self.DBG = DBG      # debug cut level (99 = full program)
        self.GPC = G // W   # graphs per core


DEFAULT_CFG = Cfg()


def _wrap_idx(lst):
    """dma_gather index layout: position j is read from row j%16, col j//16."""
    assert len(lst) % 16 == 0
    return np.tile(np.asarray(lst, np.int16).reshape(-1, 16).T, (8, 1))


# ---------------------------------------------------------------- host plan

def _plan(edge_index, batch, cfg):
    c = cfg
    batch = np.asarray(batch).astype(np.int64)
    ei = np.asarray(edge_index).astype(np.int64)
    owner = (batch // c.GPC).astype(np.int64)

    # self-edges appended
    src2 = np.concatenate([ei[0], np.arange(c.N, dtype=np.int64)])
    dst2 = np.concatenate([ei[1], np.arange(c.N, dtype=np.int64)])
    eowner = owner[dst2]

    # compact slot ranks per core
    n_real = np.zeros(c.W, np.int64)
    slot_of = np.full(c.N, -1, np.int64)
    node_lo = np.zeros(c.W + 1, np.int64)
    for ci in range(c.W):
        node_lo[ci] = np.searchsorted(batch, ci * c.GPC)
    node_lo[c.W] = c.N
    for ci in range(c.W):
        lo, hi = node_lo[ci], node_lo[ci + 1]
        n_real[ci] = hi - lo
        slot_of[lo:hi] = np.arange(hi - lo)
    S = int(((n_real.max() + 511) // 512) * 512)
    assert 2 * S <= 32767, f"S={S} too large for int16 conv2 ranges"
    nbin = S // 512
    nsub = S // 128
    nSC = S // 128
    gslot = owner * S + slot_of

    def build_conv(src_row, R):
        """src_row: per-edge source row id in the gather table (size R).
        Ranges are interleaved (pass = row % NR) so per-core locality in the
        source space cannot overload one pass.  The device view is
        table.rearrange("(q four) f -> four q f")[r] with elem_step.
        Returns common chunk structure + per-core idx/colidx arrays."""
        RSZ = -(-max(R, 1) // c.NR)
        RSZ = ((RSZ + 127) // 128) * 128
        assert RSZ <= 32767
        epass = src_row % c.NR
        # per (core, pass, subbin) edge lists
        counts = np.zeros((c.W, c.NR, nsub), np.int64)
        percore_edges = []
        for ci in range(c.W):
            m = eowner == ci
            sl = slot_of[dst2[m]]
            pr = epass[m]
            rows = src_row[m]
            sub = sl // 128
            order = np.lexsort((sl, sub, pr))
            sl, pr, rows, sub = sl[order], pr[order], rows[order], sub[order]
            np.add.at(counts[ci], (pr, sub), 1)
            percore_edges.append((sl, pr, rows, sub))
        # common chunk structure
        nch = np.maximum(1, -(-counts.max(axis=0) // 128))   # [NR, nsub]
        chunks = []     # (pass, subbin)
        seg_of = {}
        for r in range(c.NR):
            for sb in range(nsub):
                seg_of[(r, sb)] = (len(chunks), int(nch[r, sb]))
                for k in range(int(nch[r, sb])):
                    chunks.append((r, sb))
        C = len(chunks)
        POS = C * 128
        pass_pos_lo = np.zeros(c.NR + 1, np.int64)
        for r in range(c.NR):
            pass_pos_lo[r + 1] = pass_pos_lo[r] + 128 * int(nch[r].sum())
        # per-core arrays
        cores = []
        for ci in range(c.W):
            sl, pr, rows, sub = percore_edges[ci]
            idx_local = np.zeros(POS, np.int64)          # pad -> row 0 of range
            colv = np.full((128, C), PADCOL, np.float64)
            # compute position of each edge: within its (pass, subbin) segment
            seg_base = {}
            cursor = {}
            pos = 0
            for r in range(c.NR):
                for sb in range(nsub):
                    seg_base[(r, sb)] = pos
                    cursor[(r, sb)] = 0
                    pos += 128 * int(nch[r, sb])
            # vectorized-ish placement
            key = pr * nsub + sub
            # edges are sorted by (pr, sub, sl); within segment consecutive
            uniq, start_idx = np.unique(key, return_index=True)
            end_idx = np.append(start_idx[1:], len(key))
            for u, s0, s1 in zip(uniq, start_idx, end_idx):
                r, sb = int(u) // nsub, int(u) % nsub
                base = seg_base[(r, sb)]
                n = s1 - s0
                p = base + np.arange(n)
                idx_local[p] = rows[s0:s1] // c.NR
                colv[p % 128, p // 128] = sl[s0:s1] - sb * 128
            # per-pass wrapped idx arrays, concatenated into [128, POS/16]
            wrapped = [
                _wrap_idx(idx_local[pass_pos_lo[r]:pass_pos_lo[r + 1]])
                for r in range(c.NR) if pass_pos_lo[r + 1] > pass_pos_lo[r]
            ]
            idx16 = np.concatenate(wrapped, axis=1) if wrapped else np.zeros((128, 0), np.int16)
            cores.append(dict(idx16=idx16, colidx=colv))
        # gather op list: per pass, ops of <= NIMAX positions
        ops = []        # (pass, pos_lo, ni)
        for r in range(c.NR):
            p0, p1 = int(pass_pos_lo[r]), int(pass_pos_lo[r + 1])
            while p0 < p1:
                ni = min(c.NIMAX, p1 - p0)
                ops.append((r, p0, ni))
                p0 += ni
        return dict(R=R, RSZ=RSZ, C=C, POS=POS, chunks=chunks, ops=ops,
                    cores=cores, seg_of=seg_of)

    conv1 = build_conv(src2, c.N)                # gather from x rows
    conv2 = build_conv(gslot[src2], c.W * S)     # gather from h1all rows
    assert conv2["R"] <= c.W * S

    # pooling plan
    gos_all = []
    for ci in range(c.W):
        gos = np.full(S, -1, np.int64)
        lo, hi = node_lo[ci], node_lo[ci + 1]
        gos[:hi - lo] = batch[lo:hi] - ci * c.GPC
        gos_all.append(gos)
    win_lo = np.zeros(nSC, np.int64)
    prev = 0
    for k in range(nSC):
        lo_k, hi_k = c.GPC, -1
        for gos in gos_all:
            seg = gos[k * 128:(k + 1) * 128]
            v = seg[seg >= 0]
            if len(v):
                lo_k = min(lo_k, int(v.min()))
                hi_k = max(hi_k, int(v.max()))
        if hi_k < 0:
            lo_k = hi_k = min(prev, c.GPC - 1)
        assert hi_k - lo_k + 1 <= c.GW, f"pool window too wide: {lo_k}..{hi_k}"
        lo_k = max(0, min(lo_k, c.GPC - c.GW))
        assert lo_k <= prev + c.GW, "pool window coverage gap"
        win_lo[k] = lo_k
        prev = max(prev, lo_k + c.GW - 1)
    covered = np.zeros(c.GPC, bool)
    for k in range(nSC):
        covered[win_lo[k]:win_lo[k] + c.GW] = True
    assert covered.all()

    pmats = []
    for ci in range(c.W):
        pmat = np.zeros((128, nSC * c.GW), np.float32)
        gos = gos_all[ci]
        for k in range(nSC):
            seg = gos[k * 128:(k + 1) * 128]
            for p in range(128):
                if seg[p] >= 0:
                    w = int(seg[p] - win_lo[k])
                    pmat[p, k * c.GW + w] = 1.0
        pmats.append(pmat)

    return dict(S=S, nbin=nbin, nSC=nSC, win_lo=win_lo, conv=[conv1, conv2],
                n_real=n_real, pmats=pmats)


# ---------------------------------------------------------------- program

def _build(plan, cfg):
    c = cfg
    S, nbin, nSC = plan["S"], plan["nbin"], plan["nSC"]
    win_lo = plan["win_lo"]
    D, OUT, FIN, GPC = c.D, c.OUT, c.FIN, c.GPC
    rg = [list(range(c.W))]
    nG = S // 512
    GDT = c.GDT

    nc = bacc.Bacc(num_devices=c.W)

    # ---- external inputs
    xg_d = nc.dram_tensor("xg", [plan["conv"][0]["RSZ"] * c.NR, D], GDT,
                          kind="ExternalInput")
    pmat_d = nc.dram_tensor("pmat", [128, nSC * c.GW], F32, kind="ExternalInput")
    idx_d, col_d = [], []
    for li in (0, 1):
        cv = plan["conv"][li]
        idx_d.append(nc.dram_tensor(f"idx{li}", [128, cv["POS"] // 16], I16,
                                    kind="ExternalInput"))
        col_d.append(nc.dram_tensor(f"col{li}", [128, cv["C"]], GDT,
                                    kind="ExternalInput"))
    code_d = nc.dram_tensor("code", [GPC, D], F32, kind="ExternalInput")
    ident_d = nc.dram_tensor("ident", [128, 128], F32, kind="ExternalInput")
    nh_d = nc.dram_tensor("nh", [128, 1], F32, kind="ExternalInput")

    wspec = {
        "c1_w1": [D, D], "c1_b1": [D], "c1_gamma": [D], "c1_beta": [D],
        "c1_w2": [D, D], "c1_b2": [D],
        "c2_w1": [D, D], "c2_b1": [D], "c2_gamma": [D], "c2_beta": [D],
        "c2_w2": [D, D], "c2_b2": [D],
        "g_l1_w": [D, D], "g_l1_b": [D], "g_l2_w": [D, OUT], "g_l2_b": [OUT],
        "fc1_w": [D, D], "fc1_b": [D], "fc2_w": [D, D], "fc2_b": [D],
        "fc3_w": [D, OUT], "fc3_b": [OUT],
        "fin_w": [2 * OUT, FIN], "fin_b": [FIN],
    }
    wd = {k: nc.dram_tensor(k, v, F32, kind="ExternalInput") for k, v in wspec.items()}

    out_d = nc.dram_tensor("out", [FIN, GPC], F32, kind="ExternalOutput")

    # ---- internal DRAM
    h1loc_d = nc.dram_tensor("h1loc", [S, D], GDT)
    RSZ2 = plan["conv"][1]["RSZ"]
    h1all_d = nc.dram_tensor("h1all", [RSZ2 * c.NR, D], GDT, addr_space="Shared")
    ar_in = [nc.dram_tensor(f"ar{i}i", [128, 2], F32) for i in (1, 2)]
    ar_out = [nc.dram_tensor(f"ar{i}o", [128, 2], F32, addr_space="Shared")
              for i in (1, 2)]

    with tile.TileContext(nc) as tc, ExitStack() as ctx:
        const = ctx.enter_context(tc.tile_pool(name="const", bufs=1))
        work = ctx.enter_context(tc.tile_pool(name="work", bufs=3))
        wide = ctx.enter_context(tc.tile_pool(name="wide", bufs=1))
        pp = ctx.enter_context(tc.tile_pool(name="pp", bufs=2, space="PSUM"))
        pp3 = ctx.enter_context(tc.tile_pool(name="pp3", bufs=3, space="PSUM"))

        def cload(dram_ap, shape, dtype, tag):
            t = const.tile(shape, dtype, tag=tag)
            nc.sync.dma_start(out=t[:], in_=dram_ap)
            return t

        ident_s = cload(ident_d[:], [128, 128], F32, "ident")
        nh_s = cload(nh_d[:], [128, 1], F32, "nh")
        pmat_s = cload(pmat_d[:], [128, nSC * c.GW], F32, "pmat")

        ws = {}
        for k, shp in wspec.items():
            if len(shp) == 2:
                ws[k] = cload(wd[k][:], shp, F32, k)
            else:
                ws[k] = cload(wd[k][:, None], [shp[0], 1], F32, k)
        finw_hi = const.tile([OUT, FIN], F32, tag="finw_hi")
        nc.sync.dma_start(out=finw_hi[:], in_=wd["fin_w"][OUT:2 * OUT, :])

        # iota row pattern repeated (for one-hot gen), in gather dtype
        IOB = 8  # chunks per one-hot op
        iota_i = const.tile([128, IOB * 128], I32, tag="iota_i")
        nc.gpsimd.iota(iota_i[:], pattern=[[0, IOB], [1, 128]], base=0,
                       channel_multiplier=0)
        iota_s = const.tile([128, IOB * 128], GDT, tag="iota_s")
        nc.vector.tensor_copy(out=iota_s[:], in_=iota_i[:])

        ones_d1 = const.tile([OUT, 1], F32, tag="ones_d1")
        nc.vector.memset(ones_d1[:], 1.0)
        ones_1d = const.tile([1, OUT], F32, tag="ones_1d")
        nc.vector.memset(ones_1d[:], 1.0)
        ones_f1 = const.tile([FIN, 1], F32, tag="ones_f1")
        nc.vector.memset(ones_f1[:], 1.0)
        ones_1f = const.tile([1, FIN], F32, tag="ones_1f")
        nc.vector.memset(ones_1f[:], 1.0)

        # =========================== code MLP branch (fills bubbles)
        nbl = (GPC + 127) // 128
        code_nm = const.tile([128, nbl * D], F32, tag="code_nm")
        nc.sync.dma_start(
            out=code_nm[:].rearrange("p (b f) -> p b f", b=nbl),
            in_=code_d[:].rearrange("(b p) f -> p b f", p=128))
        codeT = const.tile([128, GPC], F32, tag="codeT")
        for b in range(nbl):
            tp = pp.tile([128, 128], F32, tag="tp")
            nc.tensor.transpose(out=tp[:], in_=code_nm[:, b * D:(b + 1) * D],
                                identity=ident_s[:])
            nc.vector.tensor_copy(out=codeT[:, b * 128:(b + 1) * 128], in_=tp[:])
        cps = pp3.tile([128, GPC], F32, tag="zp")
        nc.tensor.matmul(out=cps[:], lhsT=ws["fc1_w"][:], rhs=codeT[:],
                         start=True, stop=True)
        c1_s = const.tile([128, GPC], F32, tag="c1_s")
        nc.scalar.activation(out=c1_s[:], in_=cps[:], func=AF.Relu,
                             bias=ws["fc1_b"][:, :1])
        cps2 = pp3.tile([128, GPC], F32, tag="zp")
        nc.tensor.matmul(out=cps2[:], lhsT=ws["fc2_w"][:], rhs=c1_s[:],
                         start=True, stop=True)
        c2_s = const.tile([128, GPC], F32, tag="c2_s")
        nc.scalar.activation(out=c2_s[:], in_=cps2[:], func=AF.Relu,
                             bias=ws["fc2_b"][:, :1])
        cps3 = pp.tile([OUT, GPC], F32, tag="up")
        nc.tensor.matmul(out=cps3[:], lhsT=ws["fc3_w"][:], rhs=c2_s[:],
                         start=True, stop=True)
        c3_s = const.tile([OUT, GPC], F32, tag="c3_s")
        nc.scalar.activation(out=c3_s[:], in_=cps3[:], func=AF.Identity,
                             bias=ws["fc3_b"][:, :1])
        e64 = const.tile([OUT, GPC], F32, tag="e64")
        nc.scalar.activation(out=e64[:], in_=c3_s[:], func=AF.Exp)
        lsp = pp.tile([1, GPC], F32, tag="tp")
        nc.tensor.matmul(out=lsp[:], lhsT=ones_d1[:], rhs=e64[:],
                         start=True, stop=True)
        lse_s = const.tile([1, GPC], F32, tag="lse_s")
        nc.scalar.activation(out=lse_s[:], in_=lsp[:], func=AF.Ln)
        bcp = pp.tile([OUT, GPC], F32, tag="up")
        nc.tensor.matmul(out=bcp[:], lhsT=ones_1d[:], rhs=lse_s[:],
                         start=True, stop=True)
        code_embT = const.tile([OUT, GPC], F32, tag="code_embT")
        nc.vector.tensor_tensor(out=code_embT[:], in0=c3_s[:], in1=bcp[:],
                                op=ALU.subtract)

        # =========================== GIN convs
        zu_t = wide.tile([128, S], F32, tag="zu")     # z, then u, then zb (in place)
        pooled_acc = const.tile([128, GPC], F32, tag="pooled_acc")
        nc.vector.memset(pooled_acc[:], 0.0)
        ag_inst = None

        idxcol = {}
        for li, cv_ in enumerate(plan["conv"]):
            i_s = const.tile([128, cv_["POS"] // 16], I16, tag=f"idx{li+1}")
            nc.sync.dma_start(out=i_s[:], in_=idx_d[li][:])
            c_s = const.tile([128, cv_["C"]], GDT, tag=f"col{li+1}")
            nc.sync.dma_start(out=c_s[:], in_=col_d[li][:])
            idxcol[li + 1] = (i_s, c_s)

        def conv(idx, cv, src_dram, idx_dram, col_dram,
                 w1_s, b1_s, gam_s, bet_s, w2_s, b2_s, ari, aro, dep=None,
                 upto="full"):
            C, POS = cv["C"], cv["POS"]
            chunks, ops = cv["chunks"], cv["ops"]
            idx_s, col_s = idxcol[idx]
            ssum = const.tile([128, nG], F32, tag=f"ssum{idx}")
            ssq = const.tile([128, nG], F32, tag=f"ssq{idx}")

            # map chunk -> (op index, block within op)
            chunk_op = []
            for oi, (r, plo, ni) in enumerate(ops):
                for b in range(ni // 128):
                    chunk_op.append((oi, b))
            assert len(chunk_op) == C

            gtiles = {}
            stiles = {}
            cur_group = None       # (bin, pass)
            zp = None
            group_started = set()  # bins with first (copy) group done

            def close_group():
                nonlocal cur_group, zp
                if cur_group is None:
                    return
                bn = cur_group[0]
                cols = slice(bn * 512, (bn + 1) * 512)
                if bn in group_started:
                    nc.vector.tensor_tensor(out=zu_t[:, cols], in0=zu_t[:, cols],
                                            in1=zp[:], op=ALU.add)
                else:
                    nc.vector.tensor_copy(out=zu_t[:, cols], in_=zp[:])
                    group_started.add(bn)
                cur_group, zp = None, None

            for ci in range(C):
                r, sb = chunks[ci]
                bn, sl4 = sb // 4, sb % 4
                oi, blk = chunk_op[ci]
                if oi not in gtiles:
                    opr, plo, ni = ops[oi]
                    gt = work.tile([128, c.NIMAX], GDT, tag="gt")
                    src_view = src_dram[:].rearrange(
                        "(q four) f -> four q f", four=c.NR)[opr]
                    g_ins = nc.gpsimd.dma_gather(
                        gt[:, :ni].rearrange("p (k f) -> p k f", k=ni // 128),
                        src_view,
                        idx_s[:, plo // 16:(plo + ni) // 16],
                        ni, ni, 128, elem_step=c.NR * D,
                        single_packet=False)
                    if dep is not None:
                        add_dep_helper(g_ins.ins, dep.ins, True, "gather after AG")
                    gtiles = {oi: gt}
                if ci % IOB == 0:
                    nob = min(IOB, C - ci)
                    st = work.tile([128, IOB * 128], GDT, tag="st")
                    nc.vector.tensor_tensor(
                        out=st[:, :nob * 128].rearrange("p (c f) -> p c f", c=nob),
                        in0=col_s[:, ci:ci + nob].to_broadcast([128, nob, 128]),
                        in1=iota_s[:, :nob * 128].rearrange("p (c f) -> p c f", c=nob),
                        op=ALU.is_equal)
                    stiles = {ci // IOB: st}
                if cur_group != (bn, r):
                    close_group()
                    cur_group = (bn, r)
                    zp = pp3.tile([128, 512], F32, tag="zp")
                # start flag: first chunk of this (bin, pass) group
                is_first = (ci == 0 or chunks[ci - 1][0] != r
                            or chunks[ci - 1][1] // 4 != bn)
                is_last = (ci == C - 1 or chunks[ci + 1][0] != chunks[ci][0]
                           or chunks[ci + 1][1] // 4 != bn)
                nc.tensor.matmul(
                    out=zp[:, sl4 * 128:(sl4 + 1) * 128],
                    lhsT=gtiles[oi][:, blk * 128:(blk + 1) * 128],
                    rhs=stiles[ci // IOB][:, (ci % IOB) * 128:(ci % IOB + 1) * 128],
                    start=is_first, stop=is_last,
                    skip_group_check=True)
            close_group()
            if upto == "agg":
                return

            # ---- layer 1 + stats
            for g in range(nG):
                cols = slice(g * 512, (g + 1) * 512)
                up = pp.tile([128, 512], F32, tag="up")
                nc.tensor.matmul(out=up[:], lhsT=w1_s[:], rhs=zu_t[:, cols],
                                 start=True, stop=True)
                nc.scalar.activation(out=zu_t[:, cols], in_=up[:],
                                     func=AF.Identity, bias=b1_s[:, :1],
                                     accum_out=ssum[:, g:g + 1])
                sq = work.tile([128, 512], F32, tag="sq")
                nc.scalar.activation(out=sq[:], in_=zu_t[:, cols],
                                     func=AF.Square,
                                     accum_out=ssq[:, g:g + 1])

            # ---- BN stats + AllReduce
            sum_r = const.tile([128, 1], F32, tag=f"sum_r{idx}")
            ssq_r = const.tile([128, 1], F32, tag=f"ssq_r{idx}")
            nc.vector.tensor_reduce(out=sum_r[:], in_=ssum[:],
                                    axis=mybir.AxisListType.X, op=ALU.add)
            nc.vector.tensor_reduce(out=ssq_r[:], in_=ssq[:],
                                    axis=mybir.AxisListType.X, op=ALU.add)
            b1sq = const.tile([128, 1], F32, tag=f"b1sq{idx}")
            nc.scalar.activation(out=b1sq[:], in_=b1_s[:], func=AF.Square)
            tmp1 = const.tile([128, 1], F32, tag=f"tmp1_{idx}")
            nc.vector.tensor_tensor(out=tmp1[:], in0=b1_s[:], in1=nh_s[:],
                                    op=ALU.mult)
            nc.vector.tensor_tensor(out=sum_r[:], in0=sum_r[:], in1=tmp1[:],
                                    op=ALU.subtract)
            nc.vector.tensor_tensor(out=tmp1[:], in0=b1sq[:], in1=nh_s[:],
                                    op=ALU.mult)
            nc.vector.tensor_tensor(out=ssq_r[:], in0=ssq_r[:], in1=tmp1[:],
                                    op=ALU.subtract)
            if upto == "stats":
                return
            pack = const.tile([128, 2], F32, tag=f"pack{idx}")
            nc.vector.tensor_copy(out=pack[:, 0:1], in_=sum_r[:])
            nc.vector.tensor_copy(out=pack[:, 1:2], in_=ssq_r[:])
            nc.sync.dma_start(out=ari[:], in_=pack[:])
            ar = nc.gpsimd.collective_compute(
                "AllReduce", ALU.add, replica_groups=rg,
                ins=[ari[:]], outs=[aro[:]])
            rb = const.tile([128, 2], F32, tag=f"rb{idx}")
            d = nc.sync.dma_start(out=rb[:], in_=aro[:])
            add_dep_helper(d.ins, ar.ins, True, "read after AR")
            mean = const.tile([128, 1], F32, tag=f"mean{idx}")
            m2 = const.tile([128, 1], F32, tag=f"m2{idx}")
            nc.scalar.activation(out=mean[:], in_=rb[:, 0:1], func=AF.Copy,
                                 scale=1.0 / c.N)
            nc.scalar.activation(out=m2[:], in_=rb[:, 1:2], func=AF.Copy,
                                 scale=1.0 / c.N)
            msq = const.tile([128, 1], F32, tag=f"msq{idx}")
            nc.scalar.activation(out=msq[:], in_=mean[:], func=AF.Square)
            var = const.tile([128, 1], F32, tag=f"var{idx}")
            nc.vector.tensor_tensor(out=var[:], in0=m2[:], in1=msq[:],
                                    op=ALU.subtract)
            nc.vector.tensor_scalar_add(out=var[:], in0=var[:], scalar1=BN_EPS)
            std = const.tile([128, 1], F32, tag=f"std{idx}")
            nc.scalar.activation(out=std[:], in_=var[:], func=AF.Sqrt)
            inv = const.tile([128, 1], F32, tag=f"inv{idx}")
            nc.vector.reciprocal(out=inv[:], in_=std[:])
            sc = const.tile([128, 1], F32, tag=f"sc{idx}")
            nc.vector.tensor_tensor(out=sc[:], in0=gam_s[:], in1=inv[:],
                                    op=ALU.mult)
            sh = const.tile([128, 1], F32, tag=f"sh{idx}")
            nc.vector.tensor_tensor(out=sh[:], in0=mean[:], in1=sc[:],
                                    op=ALU.mult)
            nc.vector.tensor_tensor(out=sh[:], in0=bet_s[:], in1=sh[:],
                                    op=ALU.subtract)
            if upto == "bn":
                return

            # ---- BN apply + relu (in place), layer 2, transposes
            for g in range(nG):
                cols = slice(g * 512, (g + 1) * 512)
                nc.scalar.activation(out=zu_t[:, cols], in_=zu_t[:, cols],
                                     func=AF.Relu, bias=sh[:, :1],
                                     scale=sc[:, :1])
                hp = pp.tile([128, 512], F32, tag="up")
                nc.tensor.matmul(out=hp[:], lhsT=w2_s[:], rhs=zu_t[:, cols],
                                 start=True, stop=True)
                hb = work.tile([128, 512], F32, tag="hb")
                nc.scalar.activation(out=hb[:], in_=hp[:], func=AF.Relu,
                                     bias=b2_s[:, :1])
                hnm = work.tile([128, 4 * D], GDT if idx == 1 else F32, tag="hnm")
                for t in range(4):
                    tp = pp.tile([128, 128], F32, tag="tp")
                    nc.tensor.transpose(out=tp[:], in_=hb[:, t * 128:(t + 1) * 128],
                                        identity=ident_s[:])
                    nc.vector.tensor_copy(out=hnm[:, t * D:(t + 1) * D], in_=tp[:])
                    if idx == 2:
                        k = g * 4 + t
                        lo = int(win_lo[k])
                        poolw = pp.tile([128, c.GW], F32, tag="tp")
                        nc.tensor.matmul(
                            out=poolw[:],
                            lhsT=hnm[:, t * D:(t + 1) * D],
                            rhs=pmat_s[:, k * c.GW:(k + 1) * c.GW],
                            start=True, stop=True)
                        nc.vector.tensor_tensor(
                            out=pooled_acc[:, lo:lo + c.GW],
                            in0=pooled_acc[:, lo:lo + c.GW],
                            in1=poolw[:], op=ALU.add)
                if idx == 1:
                    nc.sync.dma_start(
                        out=h1loc_d[g * 512:(g + 1) * 512, :].rearrange(
                            "(b p) f -> p b f", p=128),
                        in_=hnm[:].rearrange("p (b f) -> p b f", b=4))

        cvs = plan["conv"]
        dbg = c.DBG
        upto1 = {1: "agg", 2: "stats", 3: "bn"}.get(dbg, "full")
        conv(1, cvs[0], xg_d, idx_d[0], col_d[0],
             ws["c1_w1"], ws["c1_b1"], ws["c1_gamma"], ws["c1_beta"],
             ws["c1_w2"], ws["c1_b2"], ar_in[0], ar_out[0], upto=upto1)
        if dbg >= 5:
            ag_inst = nc.gpsimd.collective_compute(
                "AllGather", ALU.bypass, replica_groups=rg,
                ins=[h1loc_d[:]], outs=[h1all_d[:]])
        if dbg >= 6:
            # conv2 gathers must run after the AllGather lands
            conv(2, cvs[1], h1all_d, idx_d[1], col_d[1],
                 ws["c2_w1"], ws["c2_b1"], ws["c2_gamma"], ws["c2_beta"],
                 ws["c2_w2"], ws["c2_b2"], ar_in[1], ar_out[1], dep=ag_inst)
        if dbg < 99:
            pout = const.tile([FIN, GPC], F32, tag="outT")
            nc.vector.tensor_copy(out=pout[:], in_=zu_t[0:FIN, 0:GPC])
            nc.sync.dma_start(out=out_d[:], in_=pout[:])
        else:
            # =========================== head
            hd1 = pp3.tile([128, GPC], F32, tag="zp")
            nc.tensor.matmul(out=hd1[:], lhsT=ws["g_l1_w"][:], rhs=pooled_acc[:],
                             start=True, stop=True)
            t_s = const.tile([128, GPC], F32, tag="t_s")
            nc.scalar.activation(out=t_s[:], in_=hd1[:], func=AF.Relu,
                                 bias=ws["g_l1_b"][:, :1])
            hd2 = pp.tile([OUT, GPC], F32, tag="up")
            nc.tensor.matmul(out=hd2[:], lhsT=ws["g_l2_w"][:], rhs=t_s[:],
                             start=True, stop=True)
            trans_embT = const.tile([OUT, GPC], F32, tag="trans_embT")
            nc.scalar.activation(out=trans_embT[:], in_=hd2[:], func=AF.Identity,
                                 bias=ws["g_l2_b"][:, :1])
            fp = pp.tile([FIN, GPC], F32, tag="tp")
            nc.tensor.matmul(out=fp[:], lhsT=ws["fin_w"][0:OUT, :],
                             rhs=code_embT[:], start=True, stop=False,
                             skip_group_check=True)
            nc.tensor.matmul(out=fp[:], lhsT=finw_hi[:],
                             rhs=trans_embT[:], start=False, stop=True,
                             skip_group_check=True)
            f_s = const.tile([FIN, GPC], F32, tag="f_s")
            nc.scalar.activation(out=f_s[:], in_=fp[:], func=AF.Identity,
                                 bias=ws["fin_b"][:, :1])
            ef = const.tile([FIN, GPC], F32, tag="ef")
            nc.scalar.activation(out=ef[:], in_=f_s[:], func=AF.Exp)
            lfp = pp.tile([1, GPC], F32, tag="up")
            nc.tensor.matmul(out=lfp[:], lhsT=ones_f1[:], rhs=ef[:],
                             start=True, stop=True)
            lf_s = const.tile([1, GPC], F32, tag="lf_s")
            nc.scalar.activation(out=lf_s[:], in_=lfp[:], func=AF.Ln)
            bfp = pp3.tile([FIN, GPC], F32, tag="zp")
            nc.tensor.matmul(out=bfp[:], lhsT=ones_1f[:], rhs=lf_s[:],
                             start=True, stop=True)
            outT = const.tile([FIN, GPC], F32, tag="outT")
            nc.vector.tensor_tensor(out=outT[:], in0=f_s[:], in1=bfp[:],
                                    op=ALU.subtract)
            nc.sync.dma_start(out=out_d[:], in_=outT[:])

    # order conv2 gathers after the AllGather
    if not nc.is_finalized():
        nc.finalize()
    return nc


# ---------------------------------------------------------------- runner

def make_in_maps(inputs, plan, cfg):
    c = cfg
    wnames = ["c1_w1", "c1_b1", "c1_gamma", "c1_beta", "c1_w2", "c1_b2",
              "c2_w1", "c2_b1", "c2_gamma", "c2_beta", "c2_w2", "c2_b2",
              "g_l1_w", "g_l1_b", "g_l2_w", "g_l2_b",
              "fc1_w", "fc1_b", "fc2_w", "fc2_b", "fc3_w", "fc3_b",
              "fin_w", "fin_b"]
    np_gdt = np.float32 if c.GDT == F32 else __import__("ml_dtypes").bfloat16
    x = np.asarray(inputs["x"], np.float32)
    R1, RSZ1 = plan["conv"][0]["R"], plan["conv"][0]["RSZ"]
    xg = np.zeros((RSZ1 * c.NR, c.D), np_gdt)
    xg[:x.shape[0]] = x.astype(np_gdt)
    code = np.ascontiguousarray(np.asarray(inputs["code_x"], np.float32))
    ident = np.eye(128, dtype=np.float32)
    in_maps = []
    for ci in range(c.W):
        m = {
            "xg": xg,
            "pmat": plan["pmats"][ci],
            "code": code[ci * c.GPC:(ci + 1) * c.GPC],
            "ident": ident,
            "nh": np.full((128, 1), float(plan["S"] - plan["n_real"][ci]),
                          np.float32),
        }
        for li in (0, 1):
            cv = plan["conv"][li]
            m[f"idx{li}"] = cv["cores"][ci]["idx16"]
            m[f"col{li}"] = cv["cores"][ci]["colidx"].astype(np_gdt)
        for k in wnames:
            m[k] = np.ascontiguousarray(np.asarray(inputs[k], np.float32))
        in_maps.append(m)
    return in_maps


_CACHE = {}


def _get_compiled(inputs, cfg):
    if "prog" not in _CACHE:
        plan = _plan(inputs["edge_index"], inputs["batch"], cfg)
        nc = _build(plan, cfg)
        _CACHE["prog"] = (plan, nc)
    return _CACHE["prog"]


def kernel(**inputs) -> np.ndarray:
    cfg = DEFAULT_CFG
    plan, nc = _get_compiled(inputs, cfg)
    in_maps = make_in_maps(inputs, plan, cfg)
    res = run_bass_kernel_spmd(nc, in_maps, core_ids=list(range(cfg.W)))
    outs = [res.results[ci]["out"].T for ci in range(cfg.W)]
    return np.ascontiguousarray(np.concatenate(outs, axis=0).astype(np.float32))


# revision 35
# speedup vs baseline: 1.5281x; 1.0144x over previous
"""Trainium2 Bass kernel for the GIN message-passing model (8 NeuronCores).

Sharding: graph partitioning.  Core c owns graphs [c*G/8, (c+1)*G/8) and the
contiguous node range of those graphs (batch is sorted), plus every edge whose
dst lands there (+ synthetic self-edges folding the GIN "+h" term into the
aggregation).  dst nodes get compact slot ranks.

Aggregation: edges are gathered with `dma_gather` (int16 indices, so the
source row space is split into 4 ranges => 4 passes).  Within a pass edges
are dst-sorted and packed into 128-position chunks aligned to 128-slot
"subbins"; a chunk's segment-sum is one matmul (gathered rows as stationary,
an on-chip-generated one-hot as moving operand) into the subbin's slice of a
512-slot "bin" PSUM bank.  Each (bin, pass) accumulates in PSUM, then one DVE
add folds it into the z accumulator in SBUF.  BatchNorm stats AllReduce; h1
is stored node-major (via PE transposes) and AllGathered for conv2's gather;
pooling is windowed one-hot matmuls; the MLP head runs feature-major.
"""

import sys

for _p in ("/opt/trn_rl_repo",):
    if _p not in sys.path:
        sys.path.insert(0, _p)

import numpy as np
from contextlib import ExitStack

import concourse.bass as bass
import concourse.bacc as bacc
import concourse.mybir as mybir
import concourse.tile as tile
from concourse.bass_utils import run_bass_kernel_spmd
from concourse.tile_rust import add_dep_helper

F32 = mybir.dt.float32
BF16 = mybir.dt.bfloat16
I32 = mybir.dt.int32
I16 = mybir.dt.int16
AF = mybir.ActivationFunctionType
ALU = mybir.AluOpType

BN_EPS = 1e-5
PADCOL = 200.0          # colidx value for pad positions (never matches 0..127)


class Cfg:
    def __init__(self, N=100000, E=500000, G=2048, D=128, OUT=64, FIN=2, W=8,
                 NR=4, NIMAX=4096, GW=32, GDT=BF16, DBG=99):
        self.N, self.E, self.G, self.D, self.OUT, self.FIN, self.W = N, E, G, D, OUT, FIN, W
        self.NR = NR        # source ranges (int16 index limit)
        self.NIMAX = NIMAX  # max positions per dma_gather
        self.GW = GW        # pooling window width (graphs)
        self.GDT = GDT      # gather dtype (bf16 or f32)
        self.DBG = DBG      # debug cut level (99 = full program)
        self.GPC = G // W   # graphs per core


DEFAULT_CFG = Cfg()


def _wrap_idx(lst):
    """dma_gather index layout: position j is read from row j%16, col j//16."""
    assert len(lst) % 16 == 0
    return np.tile(np.asarray(lst, np.int16).reshape(-1, 16).T, (8, 1))


# ---------------------------------------------------------------- host plan

def _plan(edge_index, batch, cfg):
    c = cfg
    batch = np.asarray(batch).astype(np.int64)
    ei = np.asarray(edge_index).astype(np.int64)
    owner = (batch // c.GPC).astype(np.int64)

    # self-edges appended
    src2 = np.concatenate([ei[0], np.arange(c.N, dtype=np.int64)])
    dst2 = np.concatenate([ei[1], np.arange(c.N, dtype=np.int64)])
    eowner = owner[dst2]

    # compact slot ranks per core
    n_real = np.zeros(c.W, np.int64)
    slot_of = np.full(c.N, -1, np.int64)
    node_lo = np.zeros(c.W + 1, np.int64)
    for ci in range(c.W):
        node_lo[ci] = np.searchsorted(batch, ci * c.GPC)
    node_lo[c.W] = c.N
    for ci in range(c.W):
        lo, hi = node_lo[ci], node_lo[ci + 1]
        n_real[ci] = hi - lo
        slot_of[lo:hi] = np.arange(hi - lo)
    S = int(((n_real.max() + 511) // 512) * 512)
    assert 2 * S <= 32767, f"S={S} too large for int16 conv2 ranges"
    nbin = S // 512
    nsub = S // 128
    nSC = S // 128
    gslot = owner * S + slot_of

    def build_conv(src_row, R):
        """src_row: per-edge source row id in the gather table (size R).
        Ranges are interleaved (pass = row % NR) so per-core locality in the
        source space cannot overload one pass.  The device view is
        table.rearrange("(q four) f -> four q f")[r] with elem_step.
        Returns common chunk structure + per-core idx/colidx arrays."""
        RSZ = -(-max(R, 1) // c.NR)
        RSZ = ((RSZ + 127) // 128) * 128
        assert RSZ <= 32767
        epass = src_row % c.NR
        # per (core, pass, subbin) edge lists
        counts = np.zeros((c.W, c.NR, nsub), np.int64)
        percore_edges = []
        for ci in range(c.W):
            m = eowner == ci
            sl = slot_of[dst2[m]]
            pr = epass[m]
            rows = src_row[m]
            sub = sl // 128
            order = np.lexsort((sl, sub, pr))
            sl, pr, rows, sub = sl[order], pr[order], rows[order], sub[order]
            np.add.at(counts[ci], (pr, sub), 1)
            percore_edges.append((sl, pr, rows, sub))
        # common chunk structure
        nch = np.maximum(1, -(-counts.max(axis=0) // 128))   # [NR, nsub]
        chunks = []     # (pass, subbin)
        seg_of = {}
        for r in range(c.NR):
            for sb in range(nsub):
                seg_of[(r, sb)] = (len(chunks), int(nch[r, sb]))
                for k in range(int(nch[r, sb])):
                    chunks.append((r, sb))
        C = len(chunks)
        POS = C * 128
        pass_pos_lo = np.zeros(c.NR + 1, np.int64)
        for r in range(c.NR):
            pass_pos_lo[r + 1] = pass_pos_lo[r] + 128 * int(nch[r].sum())
        # per-core arrays
        cores = []
        for ci in range(c.W):
            sl, pr, rows, sub = percore_edges[ci]
            idx_local = np.zeros(POS, np.int64)          # pad -> row 0 of range
            colv = np.full((128, C), PADCOL, np.float64)
            # compute position of each edge: within its (pass, subbin) segment
            seg_base = {}
            cursor = {}
            pos = 0
            for r in range(c.NR):
                for sb in range(nsub):
                    seg_base[(r, sb)] = pos
                    cursor[(r, sb)] = 0
                    pos += 128 * int(nch[r, sb])
            # vectorized-ish placement
            key = pr * nsub + sub
            # edges are sorted by (pr, sub, sl); within segment consecutive
            uniq, start_idx = np.unique(key, return_index=True)
            end_idx = np.append(start_idx[1:], len(key))
            for u, s0, s1 in zip(uniq, start_idx, end_idx):
                r, sb = int(u) // nsub, int(u) % nsub
                base = seg_base[(r, sb)]
                n = s1 - s0
                p = base + np.arange(n)
                idx_local[p] = rows[s0:s1] // c.NR
                colv[p % 128, p // 128] = sl[s0:s1] - sb * 128
            # per-pass wrapped idx arrays, concatenated into [128, POS/16]
            wrapped = [
                _wrap_idx(idx_local[pass_pos_lo[r]:pass_pos_lo[r + 1]])
                for r in range(c.NR) if pass_pos_lo[r + 1] > pass_pos_lo[r]
            ]
            idx16 = np.concatenate(wrapped, axis=1) if wrapped else np.zeros((128, 0), np.int16)
            cores.append(dict(idx16=idx16, colidx=colv))
        # gather op list: per pass, ops of <= NIMAX positions
        ops = []        # (pass, pos_lo, ni)
        for r in range(c.NR):
            p0, p1 = int(pass_pos_lo[r]), int(pass_pos_lo[r + 1])
            while p0 < p1:
                ni = min(c.NIMAX, p1 - p0)
                ops.append((r, p0, ni))
                p0 += ni
        return dict(R=R, RSZ=RSZ, C=C, POS=POS, chunks=chunks, ops=ops,
                    cores=cores, seg_of=seg_of)

    conv1 = build_conv(src2, c.N)                # gather from x rows
    conv2 = build_conv(gslot[src2], c.W * S)     # gather from h1all rows
    assert conv2["R"] <= c.W * S

    # pooling plan
    gos_all = []
    for ci in range(c.W):
        gos = np.full(S, -1, np.int64)
        lo, hi = node_lo[ci], node_lo[ci + 1]
        gos[:hi - lo] = batch[lo:hi] - ci * c.GPC
        gos_all.append(gos)
    win_lo = np.zeros(nSC, np.int64)
    prev = 0
    for k in range(nSC):
        lo_k, hi_k = c.GPC, -1
        for gos in gos_all:
            seg = gos[k * 128:(k + 1) * 128]
            v = seg[seg >= 0]
            if len(v):
                lo_k = min(lo_k, int(v.min()))
                hi_k = max(hi_k, int(v.max()))
        if hi_k < 0:
            lo_k = hi_k = min(prev, c.GPC - 1)
        assert hi_k - lo_k + 1 <= c.GW, f"pool window too wide: {lo_k}..{hi_k}"
        lo_k = max(0, min(lo_k, c.GPC - c.GW))
        assert lo_k <= prev + c.GW, "pool window coverage gap"
        win_lo[k] = lo_k
        prev = max(prev, lo_k + c.GW - 1)
    covered = np.zeros(c.GPC, bool)
    for k in range(nSC):
        covered[win_lo[k]:win_lo[k] + c.GW] = True
    assert covered.all()

    pmats = []
    for ci in range(c.W):
        pmat = np.zeros((128, nSC * c.GW), np.float32)
        gos = gos_all[ci]
        for k in range(nSC):
            seg = gos[k * 128:(k + 1) * 128]
            for p in range(128):
                if seg[p] >= 0:
                    w = int(seg[p] - win_lo[k])
                    pmat[p, k * c.GW + w] = 1.0
        pmats.append(pmat)

    return dict(S=S, nbin=nbin, nSC=nSC, win_lo=win_lo, conv=[conv1, conv2],
                n_real=n_real, pmats=pmats)


# ---------------------------------------------------------------- program

def _build(plan, cfg):
    c = cfg
    S, nbin, nSC = plan["S"], plan["nbin"], plan["nSC"]
    win_lo = plan["win_lo"]
    D, OUT, FIN, GPC = c.D, c.OUT, c.FIN, c.GPC
    rg = [list(range(c.W))]
    nG = S // 512
    GDT = c.GDT

    nc = bacc.Bacc(num_devices=c.W)

    # ---- external inputs
    xg_d = nc.dram_tensor("xg", [plan["conv"][0]["RSZ"] * c.NR, D], GDT,
                          kind="ExternalInput")
    pmat_d = nc.dram_tensor("pmat", [128, nSC * c.GW], F32, kind="ExternalInput")
    idx_d, col_d = [], []
    for li in (0, 1):
        cv = plan["conv"][li]
        idx_d.append(nc.dram_tensor(f"idx{li}", [128, cv["POS"] // 16], I16,
                                    kind="ExternalInput"))
        col_d.append(nc.dram_tensor(f"col{li}", [128, cv["C"]], GDT,
                                    kind="ExternalInput"))
    code_d = nc.dram_tensor("code", [GPC, D], F32, kind="ExternalInput")
    ident_d = nc.dram_tensor("ident", [128, 128], F32, kind="ExternalInput")
    nh_d = nc.dram_tensor("nh", [128, 1], F32, kind="ExternalInput")

    wspec = {
        "c1_w1": [D, D], "c1_b1": [D], "c1_gamma": [D], "c1_beta": [D],
        "c1_w2": [D, D], "c1_b2": [D],
        "c2_w1": [D, D], "c2_b1": [D], "c2_gamma": [D], "c2_beta": [D],
        "c2_w2": [D, D], "c2_b2": [D],
        "g_l1_w": [D, D], "g_l1_b": [D], "g_l2_w": [D, OUT], "g_l2_b": [OUT],
        "fc1_w": [D, D], "fc1_b": [D], "fc2_w": [D, D], "fc2_b": [D],
        "fc3_w": [D, OUT], "fc3_b": [OUT],
        "fin_w": [2 * OUT, FIN], "fin_b": [FIN],
    }
    wd = {k: nc.dram_tensor(k, v, F32, kind="ExternalInput") for k, v in wspec.items()}

    out_d = nc.dram_tensor("out", [FIN, GPC], F32, kind="ExternalOutput")

    # ---- internal DRAM
    h1loc_d = nc.dram_tensor("h1loc", [S, D], GDT)
    RSZ2 = plan["conv"][1]["RSZ"]
    h1all_d = nc.dram_tensor("h1all", [RSZ2 * c.NR, D], GDT, addr_space="Shared")
    ar_in = [nc.dram_tensor(f"ar{i}i", [128, 2], F32) for i in (1, 2)]
    ar_out = [nc.dram_tensor(f"ar{i}o", [128, 2], F32, addr_space="Shared")
              for i in (1, 2)]

    with tile.TileContext(nc) as tc, ExitStack() as ctx:
        const = ctx.enter_context(tc.tile_pool(name="const", bufs=1))
        work = ctx.enter_context(tc.tile_pool(name="work", bufs=3))
        gwork = ctx.enter_context(tc.tile_pool(name="gwork", bufs=4))
        swork = ctx.enter_context(tc.tile_pool(name="swork", bufs=6))
        wide = ctx.enter_context(tc.tile_pool(name="wide", bufs=1))
        pp = ctx.enter_context(tc.tile_pool(name="pp", bufs=2, space="PSUM"))
        pp3 = ctx.enter_context(tc.tile_pool(name="pp3", bufs=3, space="PSUM"))

        def cload(dram_ap, shape, dtype, tag):
            t = const.tile(shape, dtype, tag=tag)
            nc.sync.dma_start(out=t[:], in_=dram_ap)
            return t

        ident_s = cload(ident_d[:], [128, 128], F32, "ident")
        nh_s = cload(nh_d[:], [128, 1], F32, "nh")
        pmat_s = cload(pmat_d[:], [128, nSC * c.GW], F32, "pmat")

        ws = {}
        for k, shp in wspec.items():
            if len(shp) == 2:
                ws[k] = cload(wd[k][:], shp, F32, k)
            else:
                ws[k] = cload(wd[k][:, None], [shp[0], 1], F32, k)
        finw_hi = const.tile([OUT, FIN], F32, tag="finw_hi")
        nc.sync.dma_start(out=finw_hi[:], in_=wd["fin_w"][OUT:2 * OUT, :])

        # iota row pattern repeated (for one-hot gen), in gather dtype
        IOB = 8  # chunks per one-hot op
        iota_i = const.tile([128, IOB * 128], I32, tag="iota_i")
        nc.gpsimd.iota(iota_i[:], pattern=[[0, IOB], [1, 128]], base=0,
                       channel_multiplier=0)
        iota_s = const.tile([128, IOB * 128], GDT, tag="iota_s")
        nc.vector.tensor_copy(out=iota_s[:], in_=iota_i[:])

        ones_d1 = const.tile([OUT, 1], F32, tag="ones_d1")
        nc.vector.memset(ones_d1[:], 1.0)
        ones_1d = const.tile([1, OUT], F32, tag="ones_1d")
        nc.vector.memset(ones_1d[:], 1.0)
        ones_f1 = const.tile([FIN, 1], F32, tag="ones_f1")
        nc.vector.memset(ones_f1[:], 1.0)
        ones_1f = const.tile([1, FIN], F32, tag="ones_1f")
        nc.vector.memset(ones_1f[:], 1.0)

        # =========================== code MLP branch (fills bubbles)
        nbl = (GPC + 127) // 128
        code_nm = const.tile([128, nbl * D], F32, tag="code_nm")
        nc.sync.dma_start(
            out=code_nm[:].rearrange("p (b f) -> p b f", b=nbl),
            in_=code_d[:].rearrange("(b p) f -> p b f", p=128))
        codeT = const.tile([128, GPC], F32, tag="codeT")
        for b in range(nbl):
            tp = pp.tile([128, 128], F32, tag="tp")
            nc.tensor.transpose(out=tp[:], in_=code_nm[:, b * D:(b + 1) * D],
                                identity=ident_s[:])
            nc.vector.tensor_copy(out=codeT[:, b * 128:(b + 1) * 128], in_=tp[:])
        cps = pp3.tile([128, GPC], F32, tag="zp")
        nc.tensor.matmul(out=cps[:], lhsT=ws["fc1_w"][:], rhs=codeT[:],
                         start=True, stop=True)
        c1_s = const.tile([128, GPC], F32, tag="c1_s")
        nc.scalar.activation(out=c1_s[:], in_=cps[:], func=AF.Relu,
                             bias=ws["fc1_b"][:, :1])
        cps2 = pp3.tile([128, GPC], F32, tag="zp")
        nc.tensor.matmul(out=cps2[:], lhsT=ws["fc2_w"][:], rhs=c1_s[:],
                         start=True, stop=True)
        c2_s = const.tile([128, GPC], F32, tag="c2_s")
        nc.scalar.activation(out=c2_s[:], in_=cps2[:], func=AF.Relu,
                             bias=ws["fc2_b"][:, :1])
        cps3 = pp.tile([OUT, GPC], F32, tag="up")
        nc.tensor.matmul(out=cps3[:], lhsT=ws["fc3_w"][:], rhs=c2_s[:],
                         start=True, stop=True)
        c3_s = const.tile([OUT, GPC], F32, tag="c3_s")
        nc.scalar.activation(out=c3_s[:], in_=cps3[:], func=AF.Identity,
                             bias=ws["fc3_b"][:, :1])
        e64 = const.tile([OUT, GPC], F32, tag="e64")
        nc.scalar.activation(out=e64[:], in_=c3_s[:], func=AF.Exp)
        lsp = pp.tile([1, GPC], F32, tag="tp")
        nc.tensor.matmul(out=lsp[:], lhsT=ones_d1[:], rhs=e64[:],
                         start=True, stop=True)
        lse_s = const.tile([1, GPC], F32, tag="lse_s")
        nc.scalar.activation(out=lse_s[:], in_=lsp[:], func=AF.Ln)
        bcp = pp.tile([OUT, GPC], F32, tag="up")
        nc.tensor.matmul(out=bcp[:], lhsT=ones_1d[:], rhs=lse_s[:],
                         start=True, stop=True)
        code_embT = const.tile([OUT, GPC], F32, tag="code_embT")
        nc.vector.tensor_tensor(out=code_embT[:], in0=c3_s[:], in1=bcp[:],
                                op=ALU.subtract)

        # =========================== GIN convs
        zu_t = wide.tile([128, S], F32, tag="zu")     # z, then u, then zb (in place)
        pooled_acc = const.tile([128, GPC], F32, tag="pooled_acc")
        nc.vector.memset(pooled_acc[:], 0.0)
        ag_inst = None

        idxcol = {}
        for li, cv_ in enumerate(plan["conv"]):
            i_s = const.tile([128, cv_["POS"] // 16], I16, tag=f"idx{li+1}")
            nc.sync.dma_start(out=i_s[:], in_=idx_d[li][:])
            c_s = const.tile([128, cv_["C"]], GDT, tag=f"col{li+1}")
            nc.sync.dma_start(out=c_s[:], in_=col_d[li][:])
            idxcol[li + 1] = (i_s, c_s)

        def conv(idx, cv, src_dram, idx_dram, col_dram,
                 w1_s, b1_s, gam_s, bet_s, w2_s, b2_s, ari, aro, dep=None,
                 upto="full"):
            C, POS = cv["C"], cv["POS"]
            chunks, ops = cv["chunks"], cv["ops"]
            idx_s, col_s = idxcol[idx]
            ssum = const.tile([128, nG], F32, tag=f"ssum{idx}")
            ssq = const.tile([128, nG], F32, tag=f"ssq{idx}")

            # map chunk -> (op index, block within op)
            chunk_op = []
            for oi, (r, plo, ni) in enumerate(ops):
                for b in range(ni // 128):
                    chunk_op.append((oi, b))
            assert len(chunk_op) == C

            gtiles = {}
            stiles = {}
            cur_group = None       # (bin, pass)
            zp = None
            group_started = set()  # bins with first (copy) group done

            def close_group():
                nonlocal cur_group, zp
                if cur_group is None:
                    return
                bn = cur_group[0]
                cols = slice(bn * 512, (bn + 1) * 512)
                if bn in group_started:
                    nc.vector.tensor_tensor(out=zu_t[:, cols], in0=zu_t[:, cols],
                                            in1=zp[:], op=ALU.add)
                else:
                    nc.vector.tensor_copy(out=zu_t[:, cols], in_=zp[:])
                    group_started.add(bn)
                cur_group, zp = None, None

            for ci in range(C):
                r, sb = chunks[ci]
                bn, sl4 = sb // 4, sb % 4
                oi, blk = chunk_op[ci]
                if oi not in gtiles:
                    opr, plo, ni = ops[oi]
                    gt = gwork.tile([128, c.NIMAX], GDT, tag="gt")
                    src_view = src_dram[:].rearrange(
                        "(q four) f -> four q f", four=c.NR)[opr]
                    g_ins = nc.gpsimd.dma_gather(
                        gt[:, :ni].rearrange("p (k f) -> p k f", k=ni // 128),
                        src_view,
                        idx_s[:, plo // 16:(plo + ni) // 16],
                        ni, ni, 128, elem_step=c.NR * D,
                        single_packet=False)
                    if dep is not None:
                        add_dep_helper(g_ins.ins, dep.ins, True, "gather after AG")
                    gtiles = {oi: gt}
                if ci % IOB == 0:
                    nob = min(IOB, C - ci)
                    st = swork.tile([128, IOB * 128], GDT, tag="st")
                    nc.vector.tensor_tensor(
                        out=st[:, :nob * 128].rearrange("p (c f) -> p c f", c=nob),
                        in0=col_s[:, ci:ci + nob].to_broadcast([128, nob, 128]),
                        in1=iota_s[:, :nob * 128].rearrange("p (c f) -> p c f", c=nob),
                        op=ALU.is_equal)
                    stiles = {ci // IOB: st}
                if cur_group != (bn, r):
                    close_group()
                    cur_group = (bn, r)
                    zp = pp3.tile([128, 512], F32, tag="zp")
                # start flag: first chunk of this (bin, pass) group
                is_first = (ci == 0 or chunks[ci - 1][0] != r
                            or chunks[ci - 1][1] // 4 != bn)
                is_last = (ci == C - 1 or chunks[ci + 1][0] != chunks[ci][0]
                           or chunks[ci + 1][1] // 4 != bn)
                nc.tensor.matmul(
                    out=zp[:, sl4 * 128:(sl4 + 1) * 128],
                    lhsT=gtiles[oi][:, blk * 128:(blk + 1) * 128],
                    rhs=stiles[ci // IOB][:, (ci % IOB) * 128:(ci % IOB + 1) * 128],
                    start=is_first, stop=is_last,
                    skip_group_check=True)
            close_group()
            if upto == "agg":
                return

            # ---- layer 1 + stats
            for g in range(nG):
                cols = slice(g * 512, (g + 1) * 512)
                up = pp.tile([128, 512], F32, tag="up")
                nc.tensor.matmul(out=up[:], lhsT=w1_s[:], rhs=zu_t[:, cols],
                                 start=True, stop=True)
                nc.scalar.activation(out=zu_t[:, cols], in_=up[:],
                                     func=AF.Identity, bias=b1_s[:, :1],
                                     accum_out=ssum[:, g:g + 1])
                sq = work.tile([128, 512], F32, tag="sq")
                nc.scalar.activation(out=sq[:], in_=zu_t[:, cols],
                                     func=AF.Square,
                                     accum_out=ssq[:, g:g + 1])

            # ---- BN stats + AllReduce
            sum_r = const.tile([128, 1], F32, tag=f"sum_r{idx}")
            ssq_r = const.tile([128, 1], F32, tag=f"ssq_r{idx}")
            nc.vector.tensor_reduce(out=sum_r[:], in_=ssum[:],
                                    axis=mybir.AxisListType.X, op=ALU.add)
            nc.vector.tensor_reduce(out=ssq_r[:], in_=ssq[:],
                                    axis=mybir.AxisListType.X, op=ALU.add)
            b1sq = const.tile([128, 1], F32, tag=f"b1sq{idx}")
            nc.scalar.activation(out=b1sq[:], in_=b1_s[:], func=AF.Square)
            tmp1 = const.tile([128, 1], F32, tag=f"tmp1_{idx}")
            nc.vector.tensor_tensor(out=tmp1[:], in0=b1_s[:], in1=nh_s[:],
                                    op=ALU.mult)
            nc.vector.tensor_tensor(out=sum_r[:], in0=sum_r[:], in1=tmp1[:],
                                    op=ALU.subtract)
            nc.vector.tensor_tensor(out=tmp1[:], in0=b1sq[:], in1=nh_s[:],
                                    op=ALU.mult)
            nc.vector.tensor_tensor(out=ssq_r[:], in0=ssq_r[:], in1=tmp1[:],
                                    op=ALU.subtract)
            if upto == "stats":
                return
            pack = const.tile([128, 2], F32, tag=f"pack{idx}")
            nc.vector.tensor_copy(out=pack[:, 0:1], in_=sum_r[:])
            nc.vector.tensor_copy(out=pack[:, 1:2], in_=ssq_r[:])
            nc.sync.dma_start(out=ari[:], in_=pack[:])
            ar = nc.gpsimd.collective_compute(
                "AllReduce", ALU.add, replica_groups=rg,
                ins=[ari[:]], outs=[aro[:]])
            rb = const.tile([128, 2], F32, tag=f"rb{idx}")
            d = nc.sync.dma_start(out=rb[:], in_=aro[:])
            add_dep_helper(d.ins, ar.ins, True, "read after AR")
            mean = const.tile([128, 1], F32, tag=f"mean{idx}")
            m2 = const.tile([128, 1], F32, tag=f"m2{idx}")
            nc.scalar.activation(out=mean[:], in_=rb[:, 0:1], func=AF.Copy,
                                 scale=1.0 / c.N)
            nc.scalar.activation(out=m2[:], in_=rb[:, 1:2], func=AF.Copy,
                                 scale=1.0 / c.N)
            msq = const.tile([128, 1], F32, tag=f"msq{idx}")
            nc.scalar.activation(out=msq[:], in_=mean[:], func=AF.Square)
            var = const.tile([128, 1], F32, tag=f"var{idx}")
            nc.vector.tensor_tensor(out=var[:], in0=m2[:], in1=msq[:],
                                    op=ALU.subtract)
            nc.vector.tensor_scalar_add(out=var[:], in0=var[:], scalar1=BN_EPS)
            std = const.tile([128, 1], F32, tag=f"std{idx}")
            nc.scalar.activation(out=std[:], in_=var[:], func=AF.Sqrt)
            inv = const.tile([128, 1], F32, tag=f"inv{idx}")
            nc.vector.reciprocal(out=inv[:], in_=std[:])
            sc = const.tile([128, 1], F32, tag=f"sc{idx}")
            nc.vector.tensor_tensor(out=sc[:], in0=gam_s[:], in1=inv[:],
                                    op=ALU.mult)
            sh = const.tile([128, 1], F32, tag=f"sh{idx}")
            nc.vector.tensor_tensor(out=sh[:], in0=mean[:], in1=sc[:],
                                    op=ALU.mult)
            nc.vector.tensor_tensor(out=sh[:], in0=bet_s[:], in1=sh[:],
                                    op=ALU.subtract)
            if upto == "bn":
                return

            # ---- BN apply + relu (in place), layer 2, transposes
            for g in range(nG):
                cols = slice(g * 512, (g + 1) * 512)
                nc.scalar.activation(out=zu_t[:, cols], in_=zu_t[:, cols],
                                     func=AF.Relu, bias=sh[:, :1],
                                     scale=sc[:, :1])
                hp = pp.tile([128, 512], F32, tag="up")
                nc.tensor.matmul(out=hp[:], lhsT=w2_s[:], rhs=zu_t[:, cols],
                                 start=True, stop=True)
                hb = work.tile([128, 512], F32, tag="hb")
                nc.scalar.activation(out=hb[:], in_=hp[:], func=AF.Relu,
                                     bias=b2_s[:, :1])
                hnm = work.tile([128, 4 * D], GDT if idx == 1 else F32, tag="hnm")
                for t in range(4):
                    tp = pp.tile([128, 128], F32, tag="tp")
                    nc.tensor.transpose(out=tp[:], in_=hb[:, t * 128:(t + 1) * 128],
                                        identity=ident_s[:])
                    nc.vector.tensor_copy(out=hnm[:, t * D:(t + 1) * D], in_=tp[:])
                    if idx == 2:
                        k = g * 4 + t
                        lo = int(win_lo[k])
                        poolw = pp.tile([128, c.GW], F32, tag="tp")
                        nc.tensor.matmul(
                            out=poolw[:],
                            lhsT=hnm[:, t * D:(t + 1) * D],
                            rhs=pmat_s[:, k * c.GW:(k + 1) * c.GW],
                            start=True, stop=True)
                        nc.vector.tensor_tensor(
                            out=pooled_acc[:, lo:lo + c.GW],
                            in0=pooled_acc[:, lo:lo + c.GW],
                            in1=poolw[:], op=ALU.add)
                if idx == 1:
                    nc.sync.dma_start(
                        out=h1loc_d[g * 512:(g + 1) * 512, :].rearrange(
                            "(b p) f -> p b f", p=128),
                        in_=hnm[:].rearrange("p (b f) -> p b f", b=4))

        cvs = plan["conv"]
        dbg = c.DBG
        upto1 = {1: "agg", 2: "stats", 3: "bn"}.get(dbg, "full")
        conv(1, cvs[0], xg_d, idx_d[0], col_d[0],
             ws["c1_w1"], ws["c1_b1"], ws["c1_gamma"], ws["c1_beta"],
             ws["c1_w2"], ws["c1_b2"], ar_in[0], ar_out[0], upto=upto1)
        if dbg >= 5:
            ag_inst = nc.gpsimd.collective_compute(
                "AllGather", ALU.bypass, replica_groups=rg,
                ins=[h1loc_d[:]], outs=[h1all_d[:]])
        if dbg >= 6:
            # conv2 gathers must run after the AllGather lands
            conv(2, cvs[1], h1all_d, idx_d[1], col_d[1],
                 ws["c2_w1"], ws["c2_b1"], ws["c2_gamma"], ws["c2_beta"],
                 ws["c2_w2"], ws["c2_b2"], ar_in[1], ar_out[1], dep=ag_inst)
        if dbg < 99:
            pout = const.tile([FIN, GPC], F32, tag="outT")
            nc.vector.tensor_copy(out=pout[:], in_=zu_t[0:FIN, 0:GPC])
            nc.sync.dma_start(out=out_d[:], in_=pout[:])
        else:
            # =========================== head
            hd1 = pp3.tile([128, GPC], F32, tag="zp")
            nc.tensor.matmul(out=hd1[:], lhsT=ws["g_l1_w"][:], rhs=pooled_acc[:],
                             start=True, stop=True)
            t_s = const.tile([128, GPC], F32, tag="t_s")
            nc.scalar.activation(out=t_s[:], in_=hd1[:], func=AF.Relu,
                                 bias=ws["g_l1_b"][:, :1])
            hd2 = pp.tile([OUT, GPC], F32, tag="up")
            nc.tensor.matmul(out=hd2[:], lhsT=ws["g_l2_w"][:], rhs=t_s[:],
                             start=True, stop=True)
            trans_embT = const.tile([OUT, GPC], F32, tag="trans_embT")
            nc.scalar.activation(out=trans_embT[:], in_=hd2[:], func=AF.Identity,
                                 bias=ws["g_l2_b"][:, :1])
            fp = pp.tile([FIN, GPC], F32, tag="tp")
            nc.tensor.matmul(out=fp[:], lhsT=ws["fin_w"][0:OUT, :],
                             rhs=code_embT[:], start=True, stop=False,
                             skip_group_check=True)
            nc.tensor.matmul(out=fp[:], lhsT=finw_hi[:],
                             rhs=trans_embT[:], start=False, stop=True,
                             skip_group_check=True)
            f_s = const.tile([FIN, GPC], F32, tag="f_s")
            nc.scalar.activation(out=f_s[:], in_=fp[:], func=AF.Identity,
                                 bias=ws["fin_b"][:, :1])
            ef = const.tile([FIN, GPC], F32, tag="ef")
            nc.scalar.activation(out=ef[:], in_=f_s[:], func=AF.Exp)
            lfp = pp.tile([1, GPC], F32, tag="up")
            nc.tensor.matmul(out=lfp[:], lhsT=ones_f1[:], rhs=ef[:],
                             start=True, stop=True)
            lf_s = const.tile([1, GPC], F32, tag="lf_s")
            nc.scalar.activation(out=lf_s[:], in_=lfp[:], func=AF.Ln)
            bfp = pp3.tile([FIN, GPC], F32, tag="zp")
            nc.tensor.matmul(out=bfp[:], lhsT=ones_1f[:], rhs=lf_s[:],
                             start=True, stop=True)
            outT = const.tile([FIN, GPC], F32, tag="outT")
            nc.vector.tensor_tensor(out=outT[:], in0=f_s[:], in1=bfp[:],
                                    op=ALU.subtract)
            nc.sync.dma_start(out=out_d[:], in_=outT[:])

    # order conv2 gathers after the AllGather
    if not nc.is_finalized():
        nc.finalize()
    return nc


# ---------------------------------------------------------------- runner

def make_in_maps(inputs, plan, cfg):
    c = cfg
    wnames = ["c1_w1", "c1_b1", "c1_gamma", "c1_beta", "c1_w2", "c1_b2",
              "c2_w1", "c2_b1", "c2_gamma", "c2_beta", "c2_w2", "c2_b2",
              "g_l1_w", "g_l1_b", "g_l2_w", "g_l2_b",
              "fc1_w", "fc1_b", "fc2_w", "fc2_b", "fc3_w", "fc3_b",
              "fin_w", "fin_b"]
    np_gdt = np.float32 if c.GDT == F32 else __import__("ml_dtypes").bfloat16
    x = np.asarray(inputs["x"], np.float32)
    R1, RSZ1 = plan["conv"][0]["R"], plan["conv"][0]["RSZ"]
    xg = np.zeros((RSZ1 * c.NR, c.D), np_gdt)
    xg[:x.shape[0]] = x.astype(np_gdt)
    code = np.ascontiguousarray(np.asarray(inputs["code_x"], np.float32))
    ident = np.eye(128, dtype=np.float32)
    in_maps = []
    for ci in range(c.W):
        m = {
            "xg": xg,
            "pmat": plan["pmats"][ci],
            "code": code[ci * c.GPC:(ci + 1) * c.GPC],
            "ident": ident,
            "nh": np.full((128, 1), float(plan["S"] - plan["n_real"][ci]),
                          np.float32),
        }
        for li in (0, 1):
            cv = plan["conv"][li]
            m[f"idx{li}"] = cv["cores"][ci]["idx16"]
            m[f"col{li}"] = cv["cores"][ci]["colidx"].astype(np_gdt)
        for k in wnames:
            m[k] = np.ascontiguousarray(np.asarray(inputs[k], np.float32))
        in_maps.append(m)
    return in_maps


_CACHE = {}


def _get_compiled(inputs, cfg):
    if "prog" not in _CACHE:
        plan = _plan(inputs["edge_index"], inputs["batch"], cfg)
        nc = _build(plan, cfg)
        _CACHE["prog"] = (plan, nc)
    return _CACHE["prog"]


def kernel(**inputs) -> np.ndarray:
    cfg = DEFAULT_CFG
    plan, nc = _get_compiled(inputs, cfg)
    in_maps = make_in_maps(inputs, plan, cfg)
    res = run_bass_kernel_spmd(nc, in_maps, core_ids=list(range(cfg.W)))
    outs = [res.results[ci]["out"].T for ci in range(cfg.W)]
    return np.ascontiguousarray(np.concatenate(outs, axis=0).astype(np.float32))


# revision 37
# speedup vs baseline: 1.5537x; 1.0167x over previous
"""Trainium2 Bass kernel for the GIN message-passing model (8 NeuronCores).

Sharding: graph partitioning.  Core c owns graphs [c*G/8, (c+1)*G/8) and the
contiguous node range of those graphs (batch is sorted), plus every edge whose
dst lands there (+ synthetic self-edges folding the GIN "+h" term into the
aggregation).  dst nodes get compact slot ranks.

Aggregation: edges are gathered with `dma_gather` (int16 indices, so the
source row space is split into 4 ranges => 4 passes).  Within a pass edges
are dst-sorted and packed into 128-position chunks aligned to 128-slot
"subbins"; a chunk's segment-sum is one matmul (gathered rows as stationary,
an on-chip-generated one-hot as moving operand) into the subbin's slice of a
512-slot "bin" PSUM bank.  Each (bin, pass) accumulates in PSUM, then one DVE
add folds it into the z accumulator in SBUF.  BatchNorm stats AllReduce; h1
is stored node-major (via PE transposes) and AllGathered for conv2's gather;
pooling is windowed one-hot matmuls; the MLP head runs feature-major.
"""

import sys

for _p in ("/opt/trn_rl_repo",):
    if _p not in sys.path:
        sys.path.insert(0, _p)

import numpy as np
from contextlib import ExitStack

import concourse.bass as bass
import concourse.bacc as bacc
import concourse.mybir as mybir
import concourse.tile as tile
from concourse.bass_utils import run_bass_kernel_spmd
from concourse.tile_rust import add_dep_helper

F32 = mybir.dt.float32
BF16 = mybir.dt.bfloat16
I32 = mybir.dt.int32
I16 = mybir.dt.int16
AF = mybir.ActivationFunctionType
ALU = mybir.AluOpType

BN_EPS = 1e-5
PADCOL = 200.0          # colidx value for pad positions (never matches 0..127)


class Cfg:
    def __init__(self, N=100000, E=500000, G=2048, D=128, OUT=64, FIN=2, W=8,
                 NR=4, NIMAX=4096, GW=32, GDT=BF16, DBG=99):
        self.N, self.E, self.G, self.D, self.OUT, self.FIN, self.W = N, E, G, D, OUT, FIN, W
        self.NR = NR        # source ranges (int16 index limit)
        self.NIMAX = NIMAX  # max positions per dma_gather
        self.GW = GW        # pooling window width (graphs)
        self.GDT = GDT      # gather dtype (bf16 or f32)
        self.DBG = DBG      # debug cut level (99 = full program)
        self.GPC = G // W   # graphs per core


DEFAULT_CFG = Cfg()


def _wrap_idx(lst):
    """dma_gather index layout: position j is read from row j%16, col j//16."""
    assert len(lst) % 16 == 0
    return np.tile(np.asarray(lst, np.int16).reshape(-1, 16).T, (8, 1))


# ---------------------------------------------------------------- host plan

def _plan(edge_index, batch, cfg):
    c = cfg
    batch = np.asarray(batch).astype(np.int64)
    ei = np.asarray(edge_index).astype(np.int64)
    owner = (batch // c.GPC).astype(np.int64)

    # self-edges appended
    src2 = np.concatenate([ei[0], np.arange(c.N, dtype=np.int64)])
    dst2 = np.concatenate([ei[1], np.arange(c.N, dtype=np.int64)])
    eowner = owner[dst2]

    # compact slot ranks per core
    n_real = np.zeros(c.W, np.int64)
    slot_of = np.full(c.N, -1, np.int64)
    node_lo = np.zeros(c.W + 1, np.int64)
    for ci in range(c.W):
        node_lo[ci] = np.searchsorted(batch, ci * c.GPC)
    node_lo[c.W] = c.N
    for ci in range(c.W):
        lo, hi = node_lo[ci], node_lo[ci + 1]
        n_real[ci] = hi - lo
        slot_of[lo:hi] = np.arange(hi - lo)
    S = int(((n_real.max() + 511) // 512) * 512)
    assert 2 * S <= 32767, f"S={S} too large for int16 conv2 ranges"
    nbin = S // 512
    nsub = S // 128
    nSC = S // 128
    gslot = owner * S + slot_of

    def build_conv(src_row, R):
        """src_row: per-edge source row id in the gather table (size R).
        Ranges are interleaved (pass = row % NR) so per-core locality in the
        source space cannot overload one pass.  The device view is
        table.rearrange("(q four) f -> four q f")[r] with elem_step.
        Returns common chunk structure + per-core idx/colidx arrays."""
        RSZ = -(-max(R, 1) // c.NR)
        RSZ = ((RSZ + 127) // 128) * 128
        assert RSZ <= 32767
        epass = src_row % c.NR
        # per (core, pass, subbin) edge lists
        counts = np.zeros((c.W, c.NR, nsub), np.int64)
        percore_edges = []
        for ci in range(c.W):
            m = eowner == ci
            sl = slot_of[dst2[m]]
            pr = epass[m]
            rows = src_row[m]
            sub = sl // 128
            order = np.lexsort((sl, sub, pr))
            sl, pr, rows, sub = sl[order], pr[order], rows[order], sub[order]
            np.add.at(counts[ci], (pr, sub), 1)
            percore_edges.append((sl, pr, rows, sub))
        # common chunk structure
        nch = np.maximum(1, -(-counts.max(axis=0) // 128))   # [NR, nsub]
        chunks = []     # (pass, subbin)
        seg_of = {}
        for r in range(c.NR):
            for sb in range(nsub):
                seg_of[(r, sb)] = (len(chunks), int(nch[r, sb]))
                for k in range(int(nch[r, sb])):
                    chunks.append((r, sb))
        C = len(chunks)
        POS = C * 128
        pass_pos_lo = np.zeros(c.NR + 1, np.int64)
        for r in range(c.NR):
            pass_pos_lo[r + 1] = pass_pos_lo[r] + 128 * int(nch[r].sum())
        # per-core arrays
        cores = []
        for ci in range(c.W):
            sl, pr, rows, sub = percore_edges[ci]
            idx_local = np.zeros(POS, np.int64)          # pad -> row 0 of range
            colv = np.full((128, C), PADCOL, np.float64)
            # compute position of each edge: within its (pass, subbin) segment
            seg_base = {}
            cursor = {}
            pos = 0
            for r in range(c.NR):
                for sb in range(nsub):
                    seg_base[(r, sb)] = pos
                    cursor[(r, sb)] = 0
                    pos += 128 * int(nch[r, sb])
            # vectorized-ish placement
            key = pr * nsub + sub
            # edges are sorted by (pr, sub, sl); within segment consecutive
            uniq, start_idx = np.unique(key, return_index=True)
            end_idx = np.append(start_idx[1:], len(key))
            for u, s0, s1 in zip(uniq, start_idx, end_idx):
                r, sb = int(u) // nsub, int(u) % nsub
                base = seg_base[(r, sb)]
                n = s1 - s0
                p = base + np.arange(n)
                idx_local[p] = rows[s0:s1] // c.NR
                colv[p % 128, p // 128] = sl[s0:s1] - sb * 128
            # per-pass wrapped idx arrays, concatenated into [128, POS/16]
            wrapped = [
                _wrap_idx(idx_local[pass_pos_lo[r]:pass_pos_lo[r + 1]])
                for r in range(c.NR) if pass_pos_lo[r + 1] > pass_pos_lo[r]
            ]
            idx16 = np.concatenate(wrapped, axis=1) if wrapped else np.zeros((128, 0), np.int16)
            cores.append(dict(idx16=idx16, colidx=colv))
        # gather op list: per pass, ops of <= NIMAX positions
        ops = []        # (pass, pos_lo, ni)
        for r in range(c.NR):
            p0, p1 = int(pass_pos_lo[r]), int(pass_pos_lo[r + 1])
            while p0 < p1:
                ni = min(c.NIMAX, p1 - p0)
                ops.append((r, p0, ni))
                p0 += ni
        return dict(R=R, RSZ=RSZ, C=C, POS=POS, chunks=chunks, ops=ops,
                    cores=cores, seg_of=seg_of)

    conv1 = build_conv(src2, c.N)                # gather from x rows
    conv2 = build_conv(gslot[src2], c.W * S)     # gather from h1all rows
    assert conv2["R"] <= c.W * S

    # pooling plan
    gos_all = []
    for ci in range(c.W):
        gos = np.full(S, -1, np.int64)
        lo, hi = node_lo[ci], node_lo[ci + 1]
        gos[:hi - lo] = batch[lo:hi] - ci * c.GPC
        gos_all.append(gos)
    win_lo = np.zeros(nSC, np.int64)
    prev = 0
    for k in range(nSC):
        lo_k, hi_k = c.GPC, -1
        for gos in gos_all:
            seg = gos[k * 128:(k + 1) * 128]
            v = seg[seg >= 0]
            if len(v):
                lo_k = min(lo_k, int(v.min()))
                hi_k = max(hi_k, int(v.max()))
        if hi_k < 0:
            lo_k = hi_k = min(prev, c.GPC - 1)
        assert hi_k - lo_k + 1 <= c.GW, f"pool window too wide: {lo_k}..{hi_k}"
        lo_k = max(0, min(lo_k, c.GPC - c.GW))
        assert lo_k <= prev + c.GW, "pool window coverage gap"
        win_lo[k] = lo_k
        prev = max(prev, lo_k + c.GW - 1)
    covered = np.zeros(c.GPC, bool)
    for k in range(nSC):
        covered[win_lo[k]:win_lo[k] + c.GW] = True
    assert covered.all()

    pmats = []
    for ci in range(c.W):
        pmat = np.zeros((128, nSC * c.GW), np.float32)
        gos = gos_all[ci]
        for k in range(nSC):
            seg = gos[k * 128:(k + 1) * 128]
            for p in range(128):
                if seg[p] >= 0:
                    w = int(seg[p] - win_lo[k])
                    pmat[p, k * c.GW + w] = 1.0
        pmats.append(pmat)

    return dict(S=S, nbin=nbin, nSC=nSC, win_lo=win_lo, conv=[conv1, conv2],
                n_real=n_real, pmats=pmats)


# ---------------------------------------------------------------- program

def _build(plan, cfg):
    c = cfg
    S, nbin, nSC = plan["S"], plan["nbin"], plan["nSC"]
    win_lo = plan["win_lo"]
    D, OUT, FIN, GPC = c.D, c.OUT, c.FIN, c.GPC
    rg = [list(range(c.W))]
    nG = S // 512
    GDT = c.GDT

    nc = bacc.Bacc(num_devices=c.W)

    # ---- external inputs
    xg_d = nc.dram_tensor("xg", [plan["conv"][0]["RSZ"] * c.NR, D], GDT,
                          kind="ExternalInput")
    pmat_d = nc.dram_tensor("pmat", [128, nSC * c.GW], F32, kind="ExternalInput")
    idx_d, col_d = [], []
    for li in (0, 1):
        cv = plan["conv"][li]
        idx_d.append(nc.dram_tensor(f"idx{li}", [128, cv["POS"] // 16], I16,
                                    kind="ExternalInput"))
        col_d.append(nc.dram_tensor(f"col{li}", [128, cv["C"]], GDT,
                                    kind="ExternalInput"))
    code_d = nc.dram_tensor("code", [GPC, D], F32, kind="ExternalInput")
    ident_d = nc.dram_tensor("ident", [128, 128], F32, kind="ExternalInput")
    nh_d = nc.dram_tensor("nh", [128, 1], F32, kind="ExternalInput")

    wspec = {
        "c1_w1": [D, D], "c1_b1": [D], "c1_gamma": [D], "c1_beta": [D],
        "c1_w2": [D, D], "c1_b2": [D],
        "c2_w1": [D, D], "c2_b1": [D], "c2_gamma": [D], "c2_beta": [D],
        "c2_w2": [D, D], "c2_b2": [D],
        "g_l1_w": [D, D], "g_l1_b": [D], "g_l2_w": [D, OUT], "g_l2_b": [OUT],
        "fc1_w": [D, D], "fc1_b": [D], "fc2_w": [D, D], "fc2_b": [D],
        "fc3_w": [D, OUT], "fc3_b": [OUT],
        "fin_w": [2 * OUT, FIN], "fin_b": [FIN],
    }
    wd = {k: nc.dram_tensor(k, v, F32, kind="ExternalInput") for k, v in wspec.items()}

    out_d = nc.dram_tensor("out", [FIN, GPC], F32, kind="ExternalOutput")

    # ---- internal DRAM
    h1loc_d = nc.dram_tensor("h1loc", [S, D], GDT)
    RSZ2 = plan["conv"][1]["RSZ"]
    h1all_d = nc.dram_tensor("h1all", [RSZ2 * c.NR, D], GDT, addr_space="Shared")
    ar_in = [nc.dram_tensor(f"ar{i}i", [128, 2], F32) for i in (1, 2)]
    ar_out = [nc.dram_tensor(f"ar{i}o", [128, 2], F32, addr_space="Shared")
              for i in (1, 2)]

    with tile.TileContext(nc) as tc, ExitStack() as ctx:
        const = ctx.enter_context(tc.tile_pool(name="const", bufs=1))
        work = ctx.enter_context(tc.tile_pool(name="work", bufs=3))
        gwork = ctx.enter_context(tc.tile_pool(name="gwork", bufs=4))
        swork = ctx.enter_context(tc.tile_pool(name="swork", bufs=6))
        wide = ctx.enter_context(tc.tile_pool(name="wide", bufs=1))
        pp = ctx.enter_context(tc.tile_pool(name="pp", bufs=2, space="PSUM"))
        pp3 = ctx.enter_context(tc.tile_pool(name="pp3", bufs=3, space="PSUM"))

        def cload(dram_ap, shape, dtype, tag):
            t = const.tile(shape, dtype, tag=tag)
            nc.sync.dma_start(out=t[:], in_=dram_ap)
            return t

        ident_s = cload(ident_d[:], [128, 128], F32, "ident")
        nh_s = cload(nh_d[:], [128, 1], F32, "nh")
        pmat_s = cload(pmat_d[:], [128, nSC * c.GW], F32, "pmat")

        ws = {}
        for k, shp in wspec.items():
            if len(shp) == 2:
                ws[k] = cload(wd[k][:], shp, F32, k)
            else:
                ws[k] = cload(wd[k][:, None], [shp[0], 1], F32, k)
        finw_hi = const.tile([OUT, FIN], F32, tag="finw_hi")
        nc.sync.dma_start(out=finw_hi[:], in_=wd["fin_w"][OUT:2 * OUT, :])

        # iota row pattern repeated (for one-hot gen), in gather dtype
        IOB = 8  # chunks per one-hot op
        iota_i = const.tile([128, IOB * 128], I32, tag="iota_i")
        nc.gpsimd.iota(iota_i[:], pattern=[[0, IOB], [1, 128]], base=0,
                       channel_multiplier=0)
        iota_s = const.tile([128, IOB * 128], GDT, tag="iota_s")
        nc.vector.tensor_copy(out=iota_s[:], in_=iota_i[:])

        ones_d1 = const.tile([OUT, 1], F32, tag="ones_d1")
        nc.vector.memset(ones_d1[:], 1.0)
        ones_1d = const.tile([1, OUT], F32, tag="ones_1d")
        nc.vector.memset(ones_1d[:], 1.0)
        ones_f1 = const.tile([FIN, 1], F32, tag="ones_f1")
        nc.vector.memset(ones_f1[:], 1.0)
        ones_1f = const.tile([1, FIN], F32, tag="ones_1f")
        nc.vector.memset(ones_1f[:], 1.0)

        # =========================== code MLP branch (fills bubbles)
        nbl = (GPC + 127) // 128
        code_nm = const.tile([128, nbl * D], F32, tag="code_nm")
        nc.sync.dma_start(
            out=code_nm[:].rearrange("p (b f) -> p b f", b=nbl),
            in_=code_d[:].rearrange("(b p) f -> p b f", p=128))
        codeT = const.tile([128, GPC], F32, tag="codeT")
        for b in range(nbl):
            tp = pp.tile([128, 128], F32, tag="tp")
            nc.tensor.transpose(out=tp[:], in_=code_nm[:, b * D:(b + 1) * D],
                                identity=ident_s[:])
            nc.vector.tensor_copy(out=codeT[:, b * 128:(b + 1) * 128], in_=tp[:])
        cps = pp3.tile([128, GPC], F32, tag="zp")
        nc.tensor.matmul(out=cps[:], lhsT=ws["fc1_w"][:], rhs=codeT[:],
                         start=True, stop=True)
        c1_s = const.tile([128, GPC], F32, tag="c1_s")
        nc.scalar.activation(out=c1_s[:], in_=cps[:], func=AF.Relu,
                             bias=ws["fc1_b"][:, :1])
        cps2 = pp3.tile([128, GPC], F32, tag="zp")
        nc.tensor.matmul(out=cps2[:], lhsT=ws["fc2_w"][:], rhs=c1_s[:],
                         start=True, stop=True)
        c2_s = const.tile([128, GPC], F32, tag="c2_s")
        nc.scalar.activation(out=c2_s[:], in_=cps2[:], func=AF.Relu,
                             bias=ws["fc2_b"][:, :1])
        cps3 = pp.tile([OUT, GPC], F32, tag="up")
        nc.tensor.matmul(out=cps3[:], lhsT=ws["fc3_w"][:], rhs=c2_s[:],
                         start=True, stop=True)
        c3_s = const.tile([OUT, GPC], F32, tag="c3_s")
        nc.scalar.activation(out=c3_s[:], in_=cps3[:], func=AF.Identity,
                             bias=ws["fc3_b"][:, :1])
        e64 = const.tile([OUT, GPC], F32, tag="e64")
        nc.scalar.activation(out=e64[:], in_=c3_s[:], func=AF.Exp)
        lsp = pp.tile([1, GPC], F32, tag="tp")
        nc.tensor.matmul(out=lsp[:], lhsT=ones_d1[:], rhs=e64[:],
                         start=True, stop=True)
        lse_s = const.tile([1, GPC], F32, tag="lse_s")
        nc.scalar.activation(out=lse_s[:], in_=lsp[:], func=AF.Ln)
        bcp = pp.tile([OUT, GPC], F32, tag="up")
        nc.tensor.matmul(out=bcp[:], lhsT=ones_1d[:], rhs=lse_s[:],
                         start=True, stop=True)
        code_embT = const.tile([OUT, GPC], F32, tag="code_embT")
        nc.vector.tensor_tensor(out=code_embT[:], in0=c3_s[:], in1=bcp[:],
                                op=ALU.subtract)

        # =========================== GIN convs
        zu_t = wide.tile([128, S], F32, tag="zu")     # z, then u, then zb (in place)
        pooled_acc = const.tile([128, GPC], F32, tag="pooled_acc")
        nc.vector.memset(pooled_acc[:], 0.0)
        ag_inst = None

        idxcol = {}
        for li, cv_ in enumerate(plan["conv"]):
            i_s = const.tile([128, cv_["POS"] // 16], I16, tag=f"idx{li+1}")
            nc.sync.dma_start(out=i_s[:], in_=idx_d[li][:])
            c_s = const.tile([128, cv_["C"]], GDT, tag=f"col{li+1}")
            nc.sync.dma_start(out=c_s[:], in_=col_d[li][:])
            idxcol[li + 1] = (i_s, c_s)

        def conv(idx, cv, src_dram, idx_dram, col_dram,
                 w1_s, b1_s, gam_s, bet_s, w2_s, b2_s, ari, aro, dep=None,
                 upto="full"):
            C, POS = cv["C"], cv["POS"]
            chunks, ops = cv["chunks"], cv["ops"]
            idx_s, col_s = idxcol[idx]
            ssum = const.tile([128, nG], F32, tag=f"ssum{idx}")
            ssq = const.tile([128, nG], F32, tag=f"ssq{idx}")

            # map chunk -> (op index, block within op)
            chunk_op = []
            for oi, (r, plo, ni) in enumerate(ops):
                for b in range(ni // 128):
                    chunk_op.append((oi, b))
            assert len(chunk_op) == C

            gtiles = {}
            stiles = {}
            cur_group = None       # (bin, pass)
            zp = None
            group_started = set()  # bins with first (copy) group done

            def close_group():
                nonlocal cur_group, zp
                if cur_group is None:
                    return
                bn = cur_group[0]
                cols = slice(bn * 512, (bn + 1) * 512)
                if bn in group_started:
                    nc.vector.tensor_tensor(out=zu_t[:, cols], in0=zu_t[:, cols],
                                            in1=zp[:], op=ALU.add)
                else:
                    nc.vector.tensor_copy(out=zu_t[:, cols], in_=zp[:])
                    group_started.add(bn)
                cur_group, zp = None, None

            for ci in range(C):
                r, sb = chunks[ci]
                bn, sl4 = sb // 4, sb % 4
                oi, blk = chunk_op[ci]
                if oi not in gtiles:
                    opr, plo, ni = ops[oi]
                    gt = gwork.tile([128, c.NIMAX], GDT, tag="gt")
                    src_view = src_dram[:].rearrange(
                        "(q four) f -> four q f", four=c.NR)[opr]
                    g_ins = nc.gpsimd.dma_gather(
                        gt[:, :ni].rearrange("p (k f) -> p k f", k=ni // 128),
                        src_view,
                        idx_s[:, plo // 16:(plo + ni) // 16],
                        ni, ni, 128, elem_step=c.NR * D,
                        single_packet=False)
                    if dep is not None:
                        add_dep_helper(g_ins.ins, dep.ins, True, "gather after AG")
                    gtiles = {oi: gt}
                if ci % IOB == 0:
                    nob = min(IOB, C - ci)
                    st = swork.tile([128, IOB * 128], GDT, tag="st")
                    nc.vector.tensor_tensor(
                        out=st[:, :nob * 128].rearrange("p (c f) -> p c f", c=nob),
                        in0=col_s[:, ci:ci + nob].to_broadcast([128, nob, 128]),
                        in1=iota_s[:, :nob * 128].rearrange("p (c f) -> p c f", c=nob),
                        op=ALU.is_equal)
                    stiles = {ci // IOB: st}
                if cur_group != (bn, r):
                    close_group()
                    cur_group = (bn, r)
                    zp = pp3.tile([128, 512], F32, tag="zp")
                # start flag: first chunk of this (bin, pass) group
                is_first = (ci == 0 or chunks[ci - 1][0] != r
                            or chunks[ci - 1][1] // 4 != bn)
                is_last = (ci == C - 1 or chunks[ci + 1][0] != chunks[ci][0]
                           or chunks[ci + 1][1] // 4 != bn)
                nc.tensor.matmul(
                    out=zp[:, sl4 * 128:(sl4 + 1) * 128],
                    lhsT=gtiles[oi][:, blk * 128:(blk + 1) * 128],
                    rhs=stiles[ci // IOB][:, (ci % IOB) * 128:(ci % IOB + 1) * 128],
                    start=is_first, stop=is_last,
                    skip_group_check=True)
            close_group()
            if upto == "agg":
                return

            # ---- layer 1 + stats
            for g in range(nG):
                cols = slice(g * 512, (g + 1) * 512)
                up = pp.tile([128, 512], F32, tag="up")
                nc.tensor.matmul(out=up[:], lhsT=w1_s[:], rhs=zu_t[:, cols],
                                 start=True, stop=True)
                nc.scalar.activation(out=zu_t[:, cols], in_=up[:],
                                     func=AF.Identity, bias=b1_s[:, :1],
                                     accum_out=ssum[:, g:g + 1])
                sq = work.tile([128, 512], F32, tag="sq")
                nc.scalar.activation(out=sq[:], in_=zu_t[:, cols],
                                     func=AF.Square,
                                     accum_out=ssq[:, g:g + 1])

            # ---- BN stats + AllReduce
            sum_r = const.tile([128, 1], F32, tag=f"sum_r{idx}")
            ssq_r = const.tile([128, 1], F32, tag=f"ssq_r{idx}")
            nc.vector.tensor_reduce(out=sum_r[:], in_=ssum[:],
                                    axis=mybir.AxisListType.X, op=ALU.add)
            nc.vector.tensor_reduce(out=ssq_r[:], in_=ssq[:],
                                    axis=mybir.AxisListType.X, op=ALU.add)
            b1sq = const.tile([128, 1], F32, tag=f"b1sq{idx}")
            nc.scalar.activation(out=b1sq[:], in_=b1_s[:], func=AF.Square)
            tmp1 = const.tile([128, 1], F32, tag=f"tmp1_{idx}")
            nc.vector.tensor_tensor(out=tmp1[:], in0=b1_s[:], in1=nh_s[:],
                                    op=ALU.mult)
            nc.vector.tensor_tensor(out=sum_r[:], in0=sum_r[:], in1=tmp1[:],
                                    op=ALU.subtract)
            nc.vector.tensor_tensor(out=tmp1[:], in0=b1sq[:], in1=nh_s[:],
                                    op=ALU.mult)
            nc.vector.tensor_tensor(out=ssq_r[:], in0=ssq_r[:], in1=tmp1[:],
                                    op=ALU.subtract)
            if upto == "stats":
                return
            pack = const.tile([128, 2], F32, tag=f"pack{idx}")
            nc.vector.tensor_copy(out=pack[:, 0:1], in_=sum_r[:])
            nc.vector.tensor_copy(out=pack[:, 1:2], in_=ssq_r[:])
            nc.sync.dma_start(out=ari[:], in_=pack[:])
            ar = nc.gpsimd.collective_compute(
                "AllReduce", ALU.add, replica_groups=rg,
                ins=[ari[:]], outs=[aro[:]])
            rb = const.tile([128, 2], F32, tag=f"rb{idx}")
            d = nc.sync.dma_start(out=rb[:], in_=aro[:])
            add_dep_helper(d.ins, ar.ins, True, "read after AR")
            mean = const.tile([128, 1], F32, tag=f"mean{idx}")
            m2 = const.tile([128, 1], F32, tag=f"m2{idx}")
            nc.scalar.activation(out=mean[:], in_=rb[:, 0:1], func=AF.Copy,
                                 scale=1.0 / c.N)
            nc.scalar.activation(out=m2[:], in_=rb[:, 1:2], func=AF.Copy,
                                 scale=1.0 / c.N)
            msq = const.tile([128, 1], F32, tag=f"msq{idx}")
            nc.scalar.activation(out=msq[:], in_=mean[:], func=AF.Square)
            var = const.tile([128, 1], F32, tag=f"var{idx}")
            nc.vector.tensor_tensor(out=var[:], in0=m2[:], in1=msq[:],
                                    op=ALU.subtract)
            nc.vector.tensor_scalar_add(out=var[:], in0=var[:], scalar1=BN_EPS)
            std = const.tile([128, 1], F32, tag=f"std{idx}")
            nc.scalar.activation(out=std[:], in_=var[:], func=AF.Sqrt)
            inv = const.tile([128, 1], F32, tag=f"inv{idx}")
            nc.vector.reciprocal(out=inv[:], in_=std[:])
            sc = const.tile([128, 1], F32, tag=f"sc{idx}")
            nc.vector.tensor_tensor(out=sc[:], in0=gam_s[:], in1=inv[:],
                                    op=ALU.mult)
            sh = const.tile([128, 1], F32, tag=f"sh{idx}")
            nc.vector.tensor_tensor(out=sh[:], in0=mean[:], in1=sc[:],
                                    op=ALU.mult)
            nc.vector.tensor_tensor(out=sh[:], in0=bet_s[:], in1=sh[:],
                                    op=ALU.subtract)
            if upto == "bn":
                return

            # ---- BN apply + relu (in place), layer 2, transposes
            for g in range(nG):
                cols = slice(g * 512, (g + 1) * 512)
                nc.scalar.activation(out=zu_t[:, cols], in_=zu_t[:, cols],
                                     func=AF.Relu, bias=sh[:, :1],
                                     scale=sc[:, :1])
                hp = pp.tile([128, 512], F32, tag="up")
                nc.tensor.matmul(out=hp[:], lhsT=w2_s[:], rhs=zu_t[:, cols],
                                 start=True, stop=True)
                hb = work.tile([128, 512], F32, tag="hb")
                nc.scalar.activation(out=hb[:], in_=hp[:], func=AF.Relu,
                                     bias=b2_s[:, :1])
                hnm = work.tile([128, 4 * D], GDT if idx == 1 else F32, tag="hnm")
                for t in range(4):
                    tp = pp.tile([128, 128], F32, tag="tp")
                    nc.tensor.transpose(out=tp[:], in_=hb[:, t * 128:(t + 1) * 128],
                                        identity=ident_s[:])
                    nc.vector.tensor_copy(out=hnm[:, t * D:(t + 1) * D], in_=tp[:])
                    if idx == 2:
                        k = g * 4 + t
                        lo = int(win_lo[k])
                        poolw = pp.tile([128, c.GW], F32, tag="tp")
                        nc.tensor.matmul(
                            out=poolw[:],
                            lhsT=hnm[:, t * D:(t + 1) * D],
                            rhs=pmat_s[:, k * c.GW:(k + 1) * c.GW],
                            start=True, stop=True)
                        nc.vector.tensor_tensor(
                            out=pooled_acc[:, lo:lo + c.GW],
                            in0=pooled_acc[:, lo:lo + c.GW],
                            in1=poolw[:], op=ALU.add)
                if idx == 1:
                    nc.sync.dma_start(
                        out=h1loc_d[g * 512:(g + 1) * 512, :].rearrange(
                            "(b p) f -> p b f", p=128),
                        in_=hnm[:].rearrange("p (b f) -> p b f", b=4))

        cvs = plan["conv"]
        dbg = c.DBG
        upto1 = {1: "agg", 2: "stats", 3: "bn"}.get(dbg, "full")
        conv(1, cvs[0], xg_d, idx_d[0], col_d[0],
             ws["c1_w1"], ws["c1_b1"], ws["c1_gamma"], ws["c1_beta"],
             ws["c1_w2"], ws["c1_b2"], ar_in[0], ar_out[0], upto=upto1)
        if dbg >= 5:
            ag_inst = nc.gpsimd.collective_compute(
                "AllGather", ALU.bypass, replica_groups=rg,
                ins=[h1loc_d[:]], outs=[h1all_d[:]])
        if dbg >= 6:
            # conv2 gathers must run after the AllGather lands
            conv(2, cvs[1], h1all_d, idx_d[1], col_d[1],
                 ws["c2_w1"], ws["c2_b1"], ws["c2_gamma"], ws["c2_beta"],
                 ws["c2_w2"], ws["c2_b2"], ar_in[1], ar_out[1], dep=ag_inst)
        if dbg < 99:
            pout = const.tile([FIN, GPC], F32, tag="outT")
            nc.vector.tensor_copy(out=pout[:], in_=zu_t[0:FIN, 0:GPC])
            nc.sync.dma_start(out=out_d[:], in_=pout[:])
        else:
            # =========================== head
            hd1 = pp3.tile([128, GPC], F32, tag="zp")
            nc.tensor.matmul(out=hd1[:], lhsT=ws["g_l1_w"][:], rhs=pooled_acc[:],
                             start=True, stop=True)
            t_s = const.tile([128, GPC], F32, tag="t_s")
            nc.scalar.activation(out=t_s[:], in_=hd1[:], func=AF.Relu,
                                 bias=ws["g_l1_b"][:, :1])
            hd2 = pp.tile([OUT, GPC], F32, tag="up")
            nc.tensor.matmul(out=hd2[:], lhsT=ws["g_l2_w"][:], rhs=t_s[:],
                             start=True, stop=True)
            trans_embT = const.tile([OUT, GPC], F32, tag="trans_embT")
            nc.scalar.activation(out=trans_embT[:], in_=hd2[:], func=AF.Identity,
                                 bias=ws["g_l2_b"][:, :1])
            fp = pp.tile([FIN, GPC], F32, tag="tp")
            nc.tensor.matmul(out=fp[:], lhsT=ws["fin_w"][0:OUT, :],
                             rhs=code_embT[:], start=True, stop=False,
                             skip_group_check=True)
            nc.tensor.matmul(out=fp[:], lhsT=finw_hi[:],
                             rhs=trans_embT[:], start=False, stop=True,
                             skip_group_check=True)
            f_s = const.tile([FIN, GPC], F32, tag="f_s")
            nc.scalar.activation(out=f_s[:], in_=fp[:], func=AF.Identity,
                                 bias=ws["fin_b"][:, :1])
            ef = const.tile([FIN, GPC], F32, tag="ef")
            nc.scalar.activation(out=ef[:], in_=f_s[:], func=AF.Exp)
            lfp = pp.tile([1, GPC], F32, tag="up")
            nc.tensor.matmul(out=lfp[:], lhsT=ones_f1[:], rhs=ef[:],
                             start=True, stop=True)
            lf_s = const.tile([1, GPC], F32, tag="lf_s")
            nc.scalar.activation(out=lf_s[:], in_=lfp[:], func=AF.Ln)
            bfp = pp3.tile([FIN, GPC], F32, tag="zp")
            nc.tensor.matmul(out=bfp[:], lhsT=ones_1f[:], rhs=lf_s[:],
                             start=True, stop=True)
            outT = const.tile([FIN, GPC], F32, tag="outT")
            nc.vector.tensor_tensor(out=outT[:], in0=f_s[:], in1=bfp[:],
                                    op=ALU.subtract)
            nc.sync.dma_start(out=out_d[:], in_=outT[:])

    # order conv2 gathers after the AllGather
    if not nc.is_finalized():
        nc.finalize()
    return nc


# ---------------------------------------------------------------- runner

def make_in_maps(inputs, plan, cfg):
    c = cfg
    wnames = ["c1_w1", "c1_b1", "c1_gamma", "c1_beta", "c1_w2", "c1_b2",
              "c2_w1", "c2_b1", "c2_gamma", "c2_beta", "c2_w2", "c2_b2",
              "g_l1_w", "g_l1_b", "g_l2_w", "g_l2_b",
              "fc1_w", "fc1_b", "fc2_w", "fc2_b", "fc3_w", "fc3_b",
              "fin_w", "fin_b"]
    np_gdt = np.float32 if c.GDT == F32 else __import__("ml_dtypes").bfloat16
    x = np.asarray(inputs["x"], np.float32)
    R1, RSZ1 = plan["conv"][0]["R"], plan["conv"][0]["RSZ"]
    xg = np.zeros((RSZ1 * c.NR, c.D), np_gdt)
    xg[:x.shape[0]] = x.astype(np_gdt)
    code = np.ascontiguousarray(np.asarray(inputs["code_x"], np.float32))
    ident = np.eye(128, dtype=np.float32)
    in_maps = []
    for ci in range(c.W):
        m = {
            "xg": xg,
            "pmat": plan["pmats"][ci],
            "code": code[ci * c.GPC:(ci + 1) * c.GPC],
            "ident": ident,
            "nh": np.full((128, 1), float(plan["S"] - plan["n_real"][ci]),
                          np.float32),
        }
        for li in (0, 1):
            cv = plan["conv"][li]
            m[f"idx{li}"] = cv["cores"][ci]["idx16"]
            m[f"col{li}"] = cv["cores"][ci]["colidx"].astype(np_gdt)
        for k in wnames:
            m[k] = np.ascontiguousarray(np.asarray(inputs[k], np.float32))
        in_maps.append(m)
    return in_maps


_CACHE = {}


def _get_compiled(inputs, cfg):
    if "prog" not in _CACHE:
        plan = _plan(inputs["edge_index"], inputs["batch"], cfg)
        nc = _build(plan, cfg)
        _CACHE["prog"] = (plan, nc)
    return _CACHE["prog"]


def kernel(**inputs) -> np.ndarray:
    cfg = DEFAULT_CFG
    plan, nc = _get_compiled(inputs, cfg)
    in_maps = make_in_maps(inputs, plan, cfg)
    res = run_bass_kernel_spmd(nc, in_maps, core_ids=list(range(cfg.W)))
    outs = [res.results[ci]["out"].T for ci in range(cfg.W)]
    return np.ascontiguousarray(np.concatenate(outs, axis=0).astype(np.float32))
